# revision 43
# baseline (speedup 1.0000x reference)
"""CrossModalAttention Trainium2 kernel v2 (8 NeuronCores, SPMD, no collectives).

Reference computation (B=4, S=2048, E=512, H=8, HD=64):
  Q = q_mod @ Wq + bq ; K = k_mod @ Wk + bk ; V = v_mod @ Wv + bv   (per head)
  scores = (Q K^T / sqrt(HD)) * modal_compat[h] ; attn = softmax(scores)
  out = (attn @ V) @ Wo + bo ; LayerNorm(out + q_mod) * gamma + beta

Sharding: core c handles batch b=c//2, query-rows half=c%2 (1024 rows each).
K/V are computed per batch on both cores of a pair (duplicated, no collectives).

v2 changes vs v1:
  - bf16 matmul operands (2 cols/cycle rhs streaming; fp32 PSUM accum).
  - score matmuls row-tiled: the two heads of a pair contract over disjoint
    PE row groups (partitions 0-63 / 64-127) and run concurrently.
  - attention processed per (head-pair, query-half): score PSUM tiles
    [128,1024] double-buffered (4 banks) + attended [65,512] x2 (2 banks)
    + V/out-proj pool (2 banks) = 8 banks, so score matmuls of tile t+1
    overlap the exp of tile t (ScalarE runs at ~100% duty).
  - attnV software-pipelined one kt behind exp.
  - V projection emitted just-in-time per key-tile inside the pair-0 loop.
  - softmax denominators: GPSIMD partition_broadcast + DVE
    reciprocal_approx_fast (replaces DRAM round-trip + 8-cyc/elem divide).
  - bv/bo folded on host into the residual (attn rows sum to 1 =>
    attended@Wo + bv@Wo + bo absorbed into xq_res); modal_compat/sqrt(HD)
    folded into Wq/bq as before.
  - LayerNorm uses fused scalar_tensor_tensor ops; the beta/gamma stage
    runs on GPSIMD to shorten the DVE tail.
"""
import sys
sys.path.insert(0, "/opt/trn_rl_repo")
import numpy as np

B, S, E, H = 4, 2048, 512, 8
HD = E // H
LN_EPS = 1e-5
N_CORES = 8
T = S // 2          # query rows per core
KT = S // 128       # key tiles (16)
TT = T // 128       # out row tiles per core (8)
NPAIR = H // 2      # head pairs (4)

_CACHE = {}


class _null_ctx:
    def __enter__(self):
        return None

    def __exit__(self, *a):
        return False


BUILD_KW = dict(ln_eng="vector", ablate=("nopre",))


def build_nc(reps: int = 1, mmdt: str = "bf16", bcast: str = "gpsimd",
             ln_eng: str = "vector", recip: str = "fast", ablate: tuple = ()):
    import concourse.tile as tile
    from concourse import bacc, mybir
    import concourse.bass as bass

    f32 = mybir.dt.float32
    f32r = mybir.dt.float32r
    bf16 = mybir.dt.bfloat16
    mdt = {"bf16": bf16, "f32r": f32r}[mmdt]
    Exp = mybir.ActivationFunctionType.Exp
    Ident = mybir.ActivationFunctionType.Identity
    Alu = mybir.AluOpType

    def bias_copy(out, in_, bias):
        """PSUM->SBUF add-bias copy: on ACT (idle on HW) unless 'dvecopy'."""
        if "dvecopy" in ablate:
            nc.vector.tensor_scalar_add(out, in_, scalar1=bias)
        else:
            nc.scalar.activation(out=out, in_=in_, func=Ident, bias=bias)

    nc = bacc.Bacc("TRN2", target_bir_lowering=False, debug=False,
                   enable_asserts=True, num_devices=N_CORES)
    dram = {}
    for name, shape, dt in [
        ("xqt", (E, T), mdt), ("xkt", (E, S), mdt), ("xvt", (E, S), mdt),
        ("wq", (E, E), mdt), ("wk", (E, E), mdt), ("wv", (E, E), mdt),
        ("wo", (E, E), mdt),
        ("bq", (E,), f32), ("bk", (E,), f32),
        ("xq_res", (T, E), f32), ("gamma", (E,), f32), ("beta", (E,), f32),
    ]:
        dram[name] = nc.dram_tensor(name, shape, dt, kind="ExternalInput").ap()
    out_d = nc.dram_tensor("out", (T, E), f32, kind="ExternalOutput").ap()

    def pbcast(ap, parts):
        """AP view broadcasting partition dim (step 0) to `parts`."""
        return bass.AP(tensor=ap.tensor, offset=ap.offset,
                       ap=[[0, parts]] + list(ap.ap[1:]))

    with tile.TileContext(nc) as tc:
        with tc.tile_pool(name="consts", bufs=1) as consts, \
             tc.tile_pool(name="persist", bufs=1) as persist:
            # weights / biases / constants
            wq_sb = consts.tile([128, 4, E], mdt)
            wk_sb = consts.tile([128, 4, E], mdt)
            wv_sb = consts.tile([128, 4, E], mdt)
            wo_sb = consts.tile([128, 4, E], mdt)
            bq_sb = consts.tile([128, 4], f32)
            bk_sb = consts.tile([128, 4], f32)
            gamma_b = consts.tile([128, E], f32)
            beta_b = consts.tile([128, E], f32)
            eps_sb = consts.tile([128, 1], f32)

            # persistent activations
            if "smallexp" in ablate:
                esc_dummy = persist.tile([128, 1024], mdt)
                if mmdt == "bf16":
                    # bf16 1.0 == 0x3F80 == 16256
                    nc.gpsimd.memset(esc_dummy[:].bitcast(mybir.dt.int16), 16256)
                else:
                    nc.gpsimd.memset(esc_dummy[:].bitcast(mybir.dt.float32), 1.0)
            # input activations, split into halves so DMA completion deps
            # stay per-half (whole-tile tracking merges waits across writers)
            xqt_h = [persist.tile([128, 4, T // 2], mdt, name=f"xqt{i}")
                     for i in range(2)]
            xkt_h = [persist.tile([128, 4, S // 2], mdt, name=f"xkt{i}")
                     for i in range(2)]
            xvt_h = [persist.tile([128, 4, S // 2], mdt, name=f"xvt{i}")
                     for i in range(2)]
            qt_sb = persist.tile([128, 4, T], mdt)     # Q.T feature-major
            kt_sb = persist.tile([128, 4, S], mdt)     # K.T feature-major
            v_sb = persist.tile([128, KT, H, HD + 1], mdt)  # V tokens + ones
            att_sb = persist.tile([128, 4, T], mdt)    # attended.T normalized
            xq_res_sb = persist.tile([128, TT, E], f32)
            xstash = persist.tile([128, TT, E], f32)   # out-proj + residual
            mvst = persist.tile([128, TT, 2], f32)     # LN (mu, var) per tile
            rstdst = persist.tile([128, TT], f32)      # LN rstd per tile
            nmrst = persist.tile([128, TT], f32)       # LN -mu*rstd per tile

            nc.gpsimd.memset(eps_sb, LN_EPS)
            # ones column of V (col HD of each head group): bf16 1.0 == 0x3F80
            if mmdt == "bf16":
                nc.gpsimd.memset(
                    v_sb[:, :, :, HD:HD + 1].bitcast(mybir.dt.int16), 16256)
            else:
                nc.gpsimd.memset(v_sb[:, :, :, HD:HD + 1], 1.0)

            def body():
                sc3 = "sc2bufs" not in ablate
                with tc.tile_pool(name="sc", bufs=3 if sc3 else 2,
                                  space="PSUM") as sc, \
                     tc.tile_pool(name="atp", bufs=1, space="PSUM") as atp, \
                     _null_ctx() if sc3 else tc.tile_pool(
                         name="vp", bufs=2, space="PSUM") as vp, \
                     tc.tile_pool(name="escp", bufs=4 if sc3 else 3) as escp, \
                     tc.tile_pool(name="dnp", bufs=2) as dnp, \
                     tc.tile_pool(name="dndp", bufs=2, space="DRAM") as dndp, \
                     tc.tile_pool(name="ln", bufs=2) as ln:

                    # ---- input DMAs (ordered by first use on the critical
                    # path: Q-proj needs wq+xqt, first scores need wk+xkt h0)
                    xq_r = dram["xqt"].rearrange("(k p) t -> p k t", p=128)
                    xk_r = dram["xkt"].rearrange("(k p) t -> p k t", p=128)
                    xv_r = dram["xvt"].rearrange("(k p) t -> p k t", p=128)
                    # Q-path inputs ride the ACT HWDGE queue so they overlap
                    # the K-path loads on the SP queue (done well before the
                    # first exp needs the ACT engine itself)
                    qeng = nc.sync if "spdma" in ablate else nc.scalar
                    qeng.dma_start(xqt_h[0], xq_r[:, :, 0:512])
                    qeng.dma_start(
                        wq_sb, dram["wq"].rearrange("(k p) e -> p k e", p=128))
                    qeng.dma_start(xqt_h[1], xq_r[:, :, 512:1024])
                    nc.sync.dma_start(
                        wk_sb, dram["wk"].rearrange("(k p) e -> p k e", p=128))
                    nc.sync.dma_start(bk_sb, dram["bk"].rearrange("(m p) -> p m", p=128))
                    nc.sync.dma_start(bq_sb, dram["bq"].rearrange("(m p) -> p m", p=128))
                    nc.sync.dma_start(xkt_h[0], xk_r[:, :, 0:1024])
                    nc.sync.dma_start(
                        wv_sb, dram["wv"].rearrange("(k p) e -> p k e", p=128))
                    nc.sync.dma_start(xvt_h[0], xv_r[:, :, 0:1024])
                    nc.sync.dma_start(xkt_h[1], xk_r[:, :, 1024:2048])
                    nc.sync.dma_start(xvt_h[1], xv_r[:, :, 1024:2048])
                    nc.sync.dma_start(
                        wo_sb, dram["wo"].rearrange("(k p) e -> p k e", p=128))
                    nc.sync.dma_start(gamma_b, pbcast(dram["gamma"][None, :], 128))
                    nc.sync.dma_start(beta_b, pbcast(dram["beta"][None, :], 128))
                    nc.sync.dma_start(
                        xq_res_sb, dram["xq_res"].rearrange("(t p) e -> p t e", p=128))

                    def proj_q(p):
                        q_ps = sc.tile([128, 1024], f32, tag="s")
                        for nn in range(2):
                            for k in range(4):
                                nc.tensor.matmul(
                                    q_ps[:, 512 * nn:512 * (nn + 1)],
                                    wq_sb[:, k, 128 * p:128 * (p + 1)],
                                    xqt_h[nn][:, k, :],
                                    start=(k == 0), stop=(k == 3))
                        bias_copy(qt_sb[:, p, :], q_ps, bq_sb[:, p:p + 1])

                    def proj_kh(p, g4):
                        """K projection for pair p, 512-key group g4 (0..3)."""
                        k_ps = sc.tile([128, 1024], f32, tag="s")
                        c0, off = 512 * g4, 512 * (g4 % 2)
                        for k in range(4):
                            nc.tensor.matmul(
                                k_ps[:, 0:512],
                                wk_sb[:, k, 128 * p:128 * (p + 1)],
                                xkt_h[g4 // 2][:, k, off:off + 512],
                                start=(k == 0), stop=(k == 3))
                        bias_copy(kt_sb[:, p, c0:c0 + 512], k_ps[:, 0:512],
                                  bk_sb[:, p:p + 1])

                    def proj_k(p, g2):
                        proj_kh(p, 2 * g2)
                        proj_kh(p, 2 * g2 + 1)

                    def proj_v(tt):
                        if sc3:
                            v_full = sc.tile([128, 1024], f32, tag="s")
                            v_ps = v_full[:, 0:512]
                        else:
                            v_ps = vp.tile([128, 512], f32, tag="v")
                        off = 128 * (tt % 8)
                        for k in range(4):
                            nc.tensor.matmul(
                                v_ps, xvt_h[tt // 8][:, k, off:off + 128],
                                wv_sb[:, k, :], start=(k == 0), stop=(k == 3))
                        if "dvecopy" in ablate:
                            nc.vector.tensor_copy(
                                v_sb[:, tt, :, 0:HD],
                                v_ps.rearrange("p (h d) -> p h d", h=H))
                        else:
                            nc.scalar.activation(
                                out=v_sb[:, tt, :, 0:HD],
                                in_=v_ps.rearrange("p (h d) -> p h d", h=H),
                                func=Ident)

                    # ---- startup projections for pair 0 ----
                    proj_q(0)
                    proj_kh(0, 0)

                    pre_cell = [None]  # pre-issued kt0 score tile for next phase

                    def attention(p, c0, w, hooks, nxt=None):
                        """Attention for head pair p, query columns [c0, c0+w).
                        nxt=(p',c0',w'): pre-issue that phase's kt0 scores at
                        kt14 so its first exp follows ours with no ACT gap."""
                        at_a = atp.tile([65, 512], f32, tag="atA")
                        at_b = atp.tile([65, 512], f32, tag="atB")
                        prev = None

                        def g2(t):
                            # [128, 2, w] view: head A at col 0, head B at 512
                            # (PSUM-bank-aligned for any w <= 512)
                            return t.rearrange("p (g x) -> p g x", g=2)[:, :, 0:w]

                        def attnv(esc, kt):
                            if "fewattnv" in ablate and kt not in (0, KT - 1):
                                return  # timing probe: WRONG results
                            nc.tensor.matmul(
                                at_a[:, 0:w], v_sb[:, kt, 2 * p, :], esc[:, 0:w],
                                start=(kt == 0), stop=(kt == KT - 1))
                            nc.tensor.matmul(
                                at_b[:, 0:w], v_sb[:, kt, 2 * p + 1, :],
                                esc[:, 512:512 + w],
                                start=(kt == 0), stop=(kt == KT - 1))

                        def score_mms(pp, cc0, ww, kt):
                            s_ps = sc.tile([128, 1024], f32, tag="s")
                            if "score1mm" in ablate:  # timing probe: WRONG
                                nc.tensor.matmul(
                                    s_ps[:, 0:ww],
                                    kt_sb[:, pp, 128 * kt:128 * (kt + 1)],
                                    qt_sb[:, pp, cc0:cc0 + ww],
                                    start=True, stop=True)
                                nc.tensor.matmul(
                                    s_ps[:, 512:512 + 64],
                                    kt_sb[:, pp, 128 * kt:128 * (kt + 1)],
                                    qt_sb[:, pp, cc0:cc0 + 64],
                                    start=True, stop=True)
                                return s_ps
                            if "norowtile" in ablate:
                                nc.tensor.matmul(
                                    s_ps[:, 0:ww],
                                    kt_sb[0:64, pp, 128 * kt:128 * (kt + 1)],
                                    qt_sb[0:64, pp, cc0:cc0 + ww],
                                    start=True, stop=True)
                                nc.tensor.matmul(
                                    s_ps[:, 512:512 + ww],
                                    kt_sb[64:128, pp, 128 * kt:128 * (kt + 1)],
                                    qt_sb[64:128, pp, cc0:cc0 + ww],
                                    start=True, stop=True)
                                return s_ps
                            nc.tensor.matmul(
                                s_ps[:, 0:ww],
                                kt_sb[0:64, pp, 128 * kt:128 * (kt + 1)],
                                qt_sb[0:64, pp, cc0:cc0 + ww],
                                start=True, stop=True, tile_position=(0, 0))
                            nc.tensor.matmul(
                                s_ps[:, 512:512 + ww],
                                kt_sb[64:128, pp, 128 * kt:128 * (kt + 1)],
                                qt_sb[64:128, pp, cc0:cc0 + ww],
                                start=True, stop=True, tile_position=(64, 0))
                            return s_ps

                        for kt in range(KT):
                            if kt == 0 and pre_cell[0] is not None:
                                s_ps = pre_cell[0]
                                pre_cell[0] = None
                            else:
                                s_ps = score_mms(p, c0, w, kt)
                            esc = escp.tile([128, 1024], mdt, tag="esc")
                            if "tinyexp" in ablate:  # timing probe: WRONG
                                nc.scalar.activation(out=esc[:, 0:64],
                                                     in_=s_ps[:, 0:64], func=Exp)
                            else:
                                nc.scalar.activation(out=g2(esc), in_=g2(s_ps),
                                                     func=Exp)
                            for h in hooks.get(kt, ()):
                                h()
                            if (kt == 14 and nxt is not None
                                    and "nopre" not in ablate):
                                pre_cell[0] = score_mms(nxt[0], nxt[1], nxt[2], 0)
                            if prev is not None:
                                attnv(*prev)
                            prev = (esc, kt)
                        attnv(*prev)

                        # normalize: att = at / den ; den in row 64 (ones col)
                        den = dnp.tile([1, 1024], f32, tag="den")
                        if "dvecopy" in ablate:
                            nc.vector.tensor_copy(den[0:1, 0:w], at_a[64:65, 0:w])
                            nc.vector.tensor_copy(den[0:1, 512:512 + w],
                                                  at_b[64:65, 0:w])
                        else:
                            nc.scalar.activation(out=den[0:1, 0:w],
                                                 in_=at_a[64:65, 0:w], func=Ident)
                            nc.scalar.activation(out=den[0:1, 512:512 + w],
                                                 in_=at_b[64:65, 0:w], func=Ident)
                        r2 = dnp.tile([128, 1024], f32, tag="r2")
                        nc.gpsimd.partition_broadcast(g2(r2), g2(den))
                        r2r = dnp.tile([128, 1024], f32, tag="r2r")
                        nc.vector.reciprocal_approx_fast(out=g2(r2r), in_=g2(r2))
                        nc.vector.tensor_mul(
                            att_sb[0:64, p, c0:c0 + w], at_a[0:64, 0:w],
                            r2r[0:64, 0:w])
                        nc.vector.tensor_mul(
                            att_sb[64:128, p, c0:c0 + w], at_b[0:64, 0:w],
                            r2r[64:128, 512:512 + w])

                    def out_ln_a(tt):
                        """Out-proj matmuls + residual add + LN stats for tile tt."""
                        o_full = sc.tile([128, 1024], f32, tag="s")
                        o_ps = o_full[:, 0:512]
                        for k in range(4):
                            nc.tensor.matmul(
                                o_ps, att_sb[:, k, 128 * tt:128 * (tt + 1)],
                                wo_sb[:, k, :], start=(k == 0), stop=(k == 3))
                        nc.vector.tensor_add(xstash[:, tt, :], o_ps,
                                             xq_res_sb[:, tt, :])
                        stats = ln.tile([128, 6], f32, tag="st")
                        nc.vector.bn_stats(stats, xstash[:, tt, :])
                        nc.vector.bn_aggr(mvst[:, tt, :], stats)

                    def rsqrt_batch(g):
                        """rstd = 1/sqrt(var+eps) for tiles 4g..4g+3 on DVE only
                        (magic-constant seed + 2 Newton steps; no ACT table)."""
                        sl = slice(4 * g, 4 * g + 4)
                        i32 = mybir.dt.int32
                        v = ln.tile([128, 4], f32, tag="rv")
                        nc.vector.tensor_scalar_add(v, mvst[:, sl, 1], LN_EPS)
                        hh = ln.tile([128, 4], f32, tag="rh")
                        nc.vector.tensor_scalar_mul(hh, v, -0.5)
                        y0 = ln.tile([128, 4], f32, tag="ry")
                        nc.vector.tensor_scalar(
                            out=y0.bitcast(i32), in0=v.bitcast(i32),
                            scalar1=1, scalar2=-1,
                            op0=Alu.arith_shift_right, op1=Alu.bitwise_xor)
                        y1 = ln.tile([128, 4], f32, tag="ry1")
                        nc.vector.tensor_scalar_add(
                            y1.bitcast(i32), y0.bitcast(i32), 0x5f375a87)
                        yy = y1
                        for it in range(2):
                            a = ln.tile([128, 4], f32, tag=f"ra{it}")
                            nc.vector.tensor_mul(a, yy, yy)
                            nc.vector.tensor_mul(a, a, hh)
                            nc.vector.tensor_scalar_add(a, a, 1.5)
                            dst = rstdst[:, sl] if it == 1 else ln.tile(
                                [128, 4], f32, tag=f"rn{it}")
                            nc.vector.tensor_mul(dst, yy, a)
                            yy = dst
                        nc.vector.scalar_tensor_tensor(
                            out=nmrst[:, sl], in0=mvst[:, sl, 0], scalar=-1.0,
                            in1=rstdst[:, sl], op0=Alu.mult, op1=Alu.mult)

                    def out_ln_b(tt):
                        """y = (x - mu) * rstd * gamma + beta, then store."""
                        y = ln.tile([128, E], f32, tag="y")
                        da = ln.tile([128, 1], f32, tag="da")
                        nc.vector.affine_mul_reduce(
                            out=y, accum_out=da, in0=xstash[:, tt, :],
                            in1=gamma_b, scale=rstdst[:, tt:tt + 1],
                            bias=nmrst[:, tt:tt + 1])
                        y2 = ln.tile([128, E], f32, tag="y2")
                        eng = nc.gpsimd if ln_eng == "gpsimd" else nc.vector
                        eng.tensor_add(y2, y, beta_b)
                        nc.sync.dma_start(out_d[128 * tt:128 * (tt + 1), :], y2)

                    # ---- main loop: pairs x query-halves; the final half is
                    # split 384/128 so softmax-normalize + out_ln overlap ----
                    WA = 384
                    for p in range(NPAIR):
                        for n in range(2):
                            if p == NPAIR - 1 and n == 1:
                                break
                            hooks = {}
                            if p == 0 and n == 0:
                                # just-in-time V projection + rest of K(0)
                                for kt in range(KT):
                                    hooks[kt] = [(lambda t=kt: proj_v(t))]
                                hooks[2].append(lambda: proj_kh(0, 1))
                                hooks[6].append(lambda: proj_k(0, 1))
                            if n == 1 and p < NPAIR - 1:
                                hooks[8] = [lambda q=p + 1: proj_q(q),
                                            lambda q=p + 1: proj_k(q, 0)]
                            if n == 0 and p > 0:
                                hooks[4] = [lambda q=p: proj_k(q, 1)]
                            if p == NPAIR - 1 and n == 0:
                                nxt = (p, 512, WA)
                            elif n == 0:
                                nxt = (p, 512, 512)
                            else:
                                nxt = (p + 1, 0, 512)
                            attention(p, 512 * n, 512, hooks, nxt=nxt)
                    if "nosplit" in ablate:
                        hooksA = {}
                        for i in range(4):
                            hooksA.setdefault(7 + 2 * i, []).append(
                                lambda t=i: out_ln_a(t))
                        hooksA.setdefault(15, []).append(lambda: rsqrt_batch(0))
                        attention(NPAIR - 1, 512, 512, hooksA)
                        for tt in range(4):
                            out_ln_b(tt)
                        for tt in range(4, TT):
                            out_ln_a(tt)
                        rsqrt_batch(1)
                        for tt in range(4, TT):
                            out_ln_b(tt)
                    else:
                        hooksA = {}
                        for i in range(4):
                            # (3,0) normalize drains ~5us into quarter A
                            hooksA.setdefault(7 + 2 * i, []).append(
                                lambda t=i: out_ln_a(t))
                        attention(NPAIR - 1, 512, WA, hooksA,
                                  nxt=(NPAIR - 1, 512 + WA, 512 - WA))
                        hooksB = {0: [lambda: rsqrt_batch(0)],
                                  2: [lambda: out_ln_a(4)],
                                  4: [lambda: out_ln_b(0)],
                                  6: [lambda: out_ln_a(5)],
                                  8: [lambda: out_ln_b(1)],
                                  10: [lambda: out_ln_b(2)],
                                  12: [lambda: out_ln_a(6)],
                                  14: [lambda: out_ln_b(3)]}
                        attention(NPAIR - 1, 512 + WA, 512 - WA, hooksB)
                        out_ln_a(7)
                        rsqrt_batch(1)
                        for tt in range(4, TT):
                            out_ln_b(tt)

            if reps == 1:
                body()
            else:
                with tc.For_i(0, reps, 1):
                    body()

    nc.compile()
    return nc


def shard_inputs(query_modality, key_modality, value_modality, Wq, bq, Wk, bk,
                 Wv, bv, Wo, bo, modal_compat, ln_gamma, ln_beta):
    """Host-side prep: fold compat into Wq/bq, bv/bo into the residual,
    pre-transpose, cast matmul operands to bf16, shard per core."""
    import ml_dtypes
    f32 = np.float32
    bf = ml_dtypes.bfloat16
    q = np.asarray(query_modality, f32)
    k = np.asarray(key_modality, f32)
    v = np.asarray(value_modality, f32)
    compat = np.asarray(modal_compat, f32).reshape(H)
    scale = np.repeat(compat / np.sqrt(HD), HD).astype(f32)     # [E]
    wq_eff = (np.asarray(Wq, f32) * scale[None, :]).astype(f32)
    bq_eff = (np.asarray(bq, f32) * scale).astype(f32)
    # attn rows sum to 1 => (attended + bv) @ Wo + bo = attended @ Wo + fold
    res_fold = (np.asarray(bv, f32) @ np.asarray(Wo, f32)
                + np.asarray(bo, f32)).astype(f32)              # [E]

    shared = {
        "wq": wq_eff.astype(bf), "wk": np.asarray(Wk, f32).astype(bf),
        "wv": np.asarray(Wv, f32).astype(bf),
        "wo": np.asarray(Wo, f32).astype(bf),
        "bq": bq_eff, "bk": np.asarray(bk, f32),
        "gamma": np.asarray(ln_gamma, f32), "beta": np.asarray(ln_beta, f32),
    }
    kt_cache = {}
    in_maps = []
    for c in range(N_CORES):
        b, half = c // 2, c % 2
        sl = slice(half * T, (half + 1) * T)
        if b not in kt_cache:
            kt_cache[b] = (np.ascontiguousarray(k[b].T).astype(bf),
                           np.ascontiguousarray(v[b].T).astype(bf))
        m = dict(shared)
        m["xqt"] = np.ascontiguousarray(q[b, sl, :].T).astype(bf)
        m["xkt"], m["xvt"] = kt_cache[b]
        m["xq_res"] = np.ascontiguousarray(q[b, sl, :] + res_fold[None, :])
        in_maps.append(m)
    return in_maps


def kernel(**inputs) -> np.ndarray:
    from concourse.bass_utils import run_bass_kernel_spmd

    if "nc" not in _CACHE:
        _CACHE["nc"] = build_nc(reps=1, **BUILD_KW)
    nc = _CACHE["nc"]
    in_maps = shard_inputs(**inputs)
    res = run_bass_kernel_spmd(nc, in_maps, core_ids=list(range(N_CORES)))
    out = np.empty((B, S, E), np.float32)
    for c in range(N_CORES):
        b, half = c // 2, c % 2
        out[b, half * T:(half + 1) * T, :] = res.results[c]["out"]
    return out



# revision 44
# speedup vs baseline: 1.0505x; 1.0505x over previous
"""CrossModalAttention Trainium2 kernel v2 (8 NeuronCores, SPMD, no collectives).

Reference computation (B=4, S=2048, E=512, H=8, HD=64):
  Q = q_mod @ Wq + bq ; K = k_mod @ Wk + bk ; V = v_mod @ Wv + bv   (per head)
  scores = (Q K^T / sqrt(HD)) * modal_compat[h] ; attn = softmax(scores)
  out = (attn @ V) @ Wo + bo ; LayerNorm(out + q_mod) * gamma + beta

Sharding: core c handles batch b=c//2, query-rows half=c%2 (1024 rows each).
K/V are computed per batch on both cores of a pair (duplicated, no collectives).

v2 changes vs v1:
  - bf16 matmul operands (2 cols/cycle rhs streaming; fp32 PSUM accum).
  - score matmuls row-tiled: the two heads of a pair contract over disjoint
    PE row groups (partitions 0-63 / 64-127) and run concurrently.
  - attention processed per (head-pair, query-half): score PSUM tiles
    [128,1024] double-buffered (4 banks) + attended [65,512] x2 (2 banks)
    + V/out-proj pool (2 banks) = 8 banks, so score matmuls of tile t+1
    overlap the exp of tile t (ScalarE runs at ~100% duty).
  - attnV software-pipelined one kt behind exp.
  - V projection emitted just-in-time per key-tile inside the pair-0 loop.
  - softmax denominators: GPSIMD partition_broadcast + DVE
    reciprocal_approx_fast (replaces DRAM round-trip + 8-cyc/elem divide).
  - bv/bo folded on host into the residual (attn rows sum to 1 =>
    attended@Wo + bv@Wo + bo absorbed into xq_res); modal_compat/sqrt(HD)
    folded into Wq/bq as before.
  - LayerNorm uses fused scalar_tensor_tensor ops; the beta/gamma stage
    runs on GPSIMD to shorten the DVE tail.
"""
import sys
sys.path.insert(0, "/opt/trn_rl_repo")
import numpy as np

B, S, E, H = 4, 2048, 512, 8
HD = E // H
LN_EPS = 1e-5
N_CORES = 8
T = S // 2          # query rows per core
KT = S // 128       # key tiles (16)
TT = T // 128       # out row tiles per core (8)
NPAIR = H // 2      # head pairs (4)

_CACHE = {}


class _null_ctx:
    def __enter__(self):
        return None

    def __exit__(self, *a):
        return False


BUILD_KW = dict(ln_eng="vector", ablate=("nopre", "dvecopy"))


def build_nc(reps: int = 1, mmdt: str = "bf16", bcast: str = "gpsimd",
             ln_eng: str = "vector", recip: str = "fast", ablate: tuple = ()):
    import concourse.tile as tile
    from concourse import bacc, mybir
    import concourse.bass as bass

    f32 = mybir.dt.float32
    f32r = mybir.dt.float32r
    bf16 = mybir.dt.bfloat16
    mdt = {"bf16": bf16, "f32r": f32r}[mmdt]
    Exp = mybir.ActivationFunctionType.Exp
    Ident = mybir.ActivationFunctionType.Identity
    Alu = mybir.AluOpType

    def bias_copy(out, in_, bias):
        """PSUM->SBUF add-bias copy: on ACT (idle on HW) unless 'dvecopy'."""
        if "dvecopy" in ablate:
            nc.vector.tensor_scalar_add(out, in_, scalar1=bias)
        else:
            nc.scalar.activation(out=out, in_=in_, func=Ident, bias=bias)

    nc = bacc.Bacc("TRN2", target_bir_lowering=False, debug=False,
                   enable_asserts=True, num_devices=N_CORES)
    dram = {}
    for name, shape, dt in [
        ("xqt", (E, T), mdt), ("xkt", (E, S), mdt), ("xvt", (E, S), mdt),
        ("wq", (E, E), mdt), ("wk", (E, E), mdt), ("wv", (E, E), mdt),
        ("wo", (E, E), mdt),
        ("bq", (E,), f32), ("bk", (E,), f32),
        ("xq_res", (T, E), f32), ("gamma", (E,), f32), ("beta", (E,), f32),
    ]:
        dram[name] = nc.dram_tensor(name, shape, dt, kind="ExternalInput").ap()
    out_d = nc.dram_tensor("out", (T, E), f32, kind="ExternalOutput").ap()

    def pbcast(ap, parts):
        """AP view broadcasting partition dim (step 0) to `parts`."""
        return bass.AP(tensor=ap.tensor, offset=ap.offset,
                       ap=[[0, parts]] + list(ap.ap[1:]))

    with tile.TileContext(nc) as tc:
        with tc.tile_pool(name="consts", bufs=1) as consts, \
             tc.tile_pool(name="persist", bufs=1) as persist:
            # weights / biases / constants
            wq_sb = consts.tile([128, 4, E], mdt)
            wk_sb = consts.tile([128, 4, E], mdt)
            wv_sb = consts.tile([128, 4, E], mdt)
            wo_sb = consts.tile([128, 4, E], mdt)
            bq_sb = consts.tile([128, 4], f32)
            bk_sb = consts.tile([128, 4], f32)
            gamma_b = consts.tile([128, E], f32)
            beta_b = consts.tile([128, E], f32)
            eps_sb = consts.tile([128, 1], f32)

            # persistent activations
            if "smallexp" in ablate:
                esc_dummy = persist.tile([128, 1024], mdt)
                if mmdt == "bf16":
                    # bf16 1.0 == 0x3F80 == 16256
                    nc.gpsimd.memset(esc_dummy[:].bitcast(mybir.dt.int16), 16256)
                else:
                    nc.gpsimd.memset(esc_dummy[:].bitcast(mybir.dt.float32), 1.0)
            # input activations, split into halves so DMA completion deps
            # stay per-half (whole-tile tracking merges waits across writers)
            xqt_h = [persist.tile([128, 4, T // 2], mdt, name=f"xqt{i}")
                     for i in range(2)]
            xkt_h = [persist.tile([128, 4, S // 2], mdt, name=f"xkt{i}")
                     for i in range(2)]
            xvt_h = [persist.tile([128, 4, S // 2], mdt, name=f"xvt{i}")
                     for i in range(2)]
            qt_sb = persist.tile([128, 4, T], mdt)     # Q.T feature-major
            kt_sb = persist.tile([128, 4, S], mdt)     # K.T feature-major
            v_sb = persist.tile([128, KT, H, HD + 1], mdt)  # V tokens + ones
            att_sb = persist.tile([128, 4, T], mdt)    # attended.T normalized
            xq_res_sb = persist.tile([128, TT, E], f32)
            xstash = persist.tile([128, TT, E], f32)   # out-proj + residual
            mvst = persist.tile([128, TT, 2], f32)     # LN (mu, var) per tile
            rstdst = persist.tile([128, TT], f32)      # LN rstd per tile
            nmrst = persist.tile([128, TT], f32)       # LN -mu*rstd per tile

            nc.gpsimd.memset(eps_sb, LN_EPS)
            # ones column of V (col HD of each head group): bf16 1.0 == 0x3F80
            if mmdt == "bf16":
                nc.gpsimd.memset(
                    v_sb[:, :, :, HD:HD + 1].bitcast(mybir.dt.int16), 16256)
            else:
                nc.gpsimd.memset(v_sb[:, :, :, HD:HD + 1], 1.0)

            def body():
                sc3 = "sc2bufs" not in ablate
                with tc.tile_pool(name="sc", bufs=3 if sc3 else 2,
                                  space="PSUM") as sc, \
                     tc.tile_pool(name="atp", bufs=1, space="PSUM") as atp, \
                     _null_ctx() if sc3 else tc.tile_pool(
                         name="vp", bufs=2, space="PSUM") as vp, \
                     tc.tile_pool(name="escp", bufs=4 if sc3 else 3) as escp, \
                     tc.tile_pool(name="dnp", bufs=2) as dnp, \
                     tc.tile_pool(name="dndp", bufs=2, space="DRAM") as dndp, \
                     tc.tile_pool(name="ln", bufs=2) as ln:

                    # ---- input DMAs (ordered by first use on the critical
                    # path: Q-proj needs wq+xqt, first scores need wk+xkt h0)
                    xq_r = dram["xqt"].rearrange("(k p) t -> p k t", p=128)
                    xk_r = dram["xkt"].rearrange("(k p) t -> p k t", p=128)
                    xv_r = dram["xvt"].rearrange("(k p) t -> p k t", p=128)
                    # Q-path inputs ride the ACT HWDGE queue so they overlap
                    # the K-path loads on the SP queue (done well before the
                    # first exp needs the ACT engine itself)
                    qeng = nc.sync if "spdma" in ablate else nc.scalar
                    qeng.dma_start(xqt_h[0], xq_r[:, :, 0:512])
                    qeng.dma_start(
                        wq_sb, dram["wq"].rearrange("(k p) e -> p k e", p=128))
                    qeng.dma_start(xqt_h[1], xq_r[:, :, 512:1024])
                    nc.sync.dma_start(
                        wk_sb, dram["wk"].rearrange("(k p) e -> p k e", p=128))
                    nc.sync.dma_start(bk_sb, dram["bk"].rearrange("(m p) -> p m", p=128))
                    nc.sync.dma_start(bq_sb, dram["bq"].rearrange("(m p) -> p m", p=128))
                    nc.sync.dma_start(xkt_h[0], xk_r[:, :, 0:1024])
                    nc.sync.dma_start(
                        wv_sb, dram["wv"].rearrange("(k p) e -> p k e", p=128))
                    nc.sync.dma_start(xvt_h[0], xv_r[:, :, 0:1024])
                    nc.sync.dma_start(xkt_h[1], xk_r[:, :, 1024:2048])
                    nc.sync.dma_start(xvt_h[1], xv_r[:, :, 1024:2048])
                    nc.sync.dma_start(
                        wo_sb, dram["wo"].rearrange("(k p) e -> p k e", p=128))
                    nc.sync.dma_start(gamma_b, pbcast(dram["gamma"][None, :], 128))
                    nc.sync.dma_start(beta_b, pbcast(dram["beta"][None, :], 128))
                    nc.sync.dma_start(
                        xq_res_sb, dram["xq_res"].rearrange("(t p) e -> p t e", p=128))

                    def proj_q(p):
                        q_ps = sc.tile([128, 1024], f32, tag="s")
                        for nn in range(2):
                            for k in range(4):
                                nc.tensor.matmul(
                                    q_ps[:, 512 * nn:512 * (nn + 1)],
                                    wq_sb[:, k, 128 * p:128 * (p + 1)],
                                    xqt_h[nn][:, k, :],
                                    start=(k == 0), stop=(k == 3))
                        bias_copy(qt_sb[:, p, :], q_ps, bq_sb[:, p:p + 1])

                    def proj_kh(p, g4):
                        """K projection for pair p, 512-key group g4 (0..3)."""
                        k_ps = sc.tile([128, 1024], f32, tag="s")
                        c0, off = 512 * g4, 512 * (g4 % 2)
                        for k in range(4):
                            nc.tensor.matmul(
                                k_ps[:, 0:512],
                                wk_sb[:, k, 128 * p:128 * (p + 1)],
                                xkt_h[g4 // 2][:, k, off:off + 512],
                                start=(k == 0), stop=(k == 3))
                        bias_copy(kt_sb[:, p, c0:c0 + 512], k_ps[:, 0:512],
                                  bk_sb[:, p:p + 1])

                    def proj_k(p, g2):
                        proj_kh(p, 2 * g2)
                        proj_kh(p, 2 * g2 + 1)

                    def proj_v(tt):
                        if sc3:
                            v_full = sc.tile([128, 1024], f32, tag="s")
                            v_ps = v_full[:, 0:512]
                        else:
                            v_ps = vp.tile([128, 512], f32, tag="v")
                        off = 128 * (tt % 8)
                        for k in range(4):
                            nc.tensor.matmul(
                                v_ps, xvt_h[tt // 8][:, k, off:off + 128],
                                wv_sb[:, k, :], start=(k == 0), stop=(k == 3))
                        if "dvecopy" in ablate:
                            nc.vector.tensor_copy(
                                v_sb[:, tt, :, 0:HD],
                                v_ps.rearrange("p (h d) -> p h d", h=H))
                        else:
                            nc.scalar.activation(
                                out=v_sb[:, tt, :, 0:HD],
                                in_=v_ps.rearrange("p (h d) -> p h d", h=H),
                                func=Ident)

                    # ---- startup projections for pair 0 ----
                    proj_q(0)
                    proj_kh(0, 0)

                    pre_cell = [None]  # pre-issued kt0 score tile for next phase

                    def attention(p, c0, w, hooks, nxt=None):
                        """Attention for head pair p, query columns [c0, c0+w).
                        nxt=(p',c0',w'): pre-issue that phase's kt0 scores at
                        kt14 so its first exp follows ours with no ACT gap."""
                        at_a = atp.tile([65, 512], f32, tag="atA")
                        at_b = atp.tile([65, 512], f32, tag="atB")
                        prev = None

                        def g2(t):
                            # [128, 2, w] view: head A at col 0, head B at 512
                            # (PSUM-bank-aligned for any w <= 512)
                            return t.rearrange("p (g x) -> p g x", g=2)[:, :, 0:w]

                        def attnv(esc, kt):
                            if "fewattnv" in ablate and kt not in (0, KT - 1):
                                return  # timing probe: WRONG results
                            nc.tensor.matmul(
                                at_a[:, 0:w], v_sb[:, kt, 2 * p, :], esc[:, 0:w],
                                start=(kt == 0), stop=(kt == KT - 1))
                            nc.tensor.matmul(
                                at_b[:, 0:w], v_sb[:, kt, 2 * p + 1, :],
                                esc[:, 512:512 + w],
                                start=(kt == 0), stop=(kt == KT - 1))

                        def score_mms(pp, cc0, ww, kt):
                            s_ps = sc.tile([128, 1024], f32, tag="s")
                            if "score1mm" in ablate:  # timing probe: WRONG
                                nc.tensor.matmul(
                                    s_ps[:, 0:ww],
                                    kt_sb[:, pp, 128 * kt:128 * (kt + 1)],
                                    qt_sb[:, pp, cc0:cc0 + ww],
                                    start=True, stop=True)
                                nc.tensor.matmul(
                                    s_ps[:, 512:512 + 64],
                                    kt_sb[:, pp, 128 * kt:128 * (kt + 1)],
                                    qt_sb[:, pp, cc0:cc0 + 64],
                                    start=True, stop=True)
                                return s_ps
                            if "norowtile" in ablate:
                                nc.tensor.matmul(
                                    s_ps[:, 0:ww],
                                    kt_sb[0:64, pp, 128 * kt:128 * (kt + 1)],
                                    qt_sb[0:64, pp, cc0:cc0 + ww],
                                    start=True, stop=True)
                                nc.tensor.matmul(
                                    s_ps[:, 512:512 + ww],
                                    kt_sb[64:128, pp, 128 * kt:128 * (kt + 1)],
                                    qt_sb[64:128, pp, cc0:cc0 + ww],
                                    start=True, stop=True)
                                return s_ps
                            nc.tensor.matmul(
                                s_ps[:, 0:ww],
                                kt_sb[0:64, pp, 128 * kt:128 * (kt + 1)],
                                qt_sb[0:64, pp, cc0:cc0 + ww],
                                start=True, stop=True, tile_position=(0, 0))
                            nc.tensor.matmul(
                                s_ps[:, 512:512 + ww],
                                kt_sb[64:128, pp, 128 * kt:128 * (kt + 1)],
                                qt_sb[64:128, pp, cc0:cc0 + ww],
                                start=True, stop=True, tile_position=(64, 0))
                            return s_ps

                        for kt in range(KT):
                            if kt == 0 and pre_cell[0] is not None:
                                s_ps = pre_cell[0]
                                pre_cell[0] = None
                            else:
                                s_ps = score_mms(p, c0, w, kt)
                            esc = escp.tile([128, 1024], mdt, tag="esc")
                            if "tinyexp" in ablate:  # timing probe: WRONG
                                nc.scalar.activation(out=esc[:, 0:64],
                                                     in_=s_ps[:, 0:64], func=Exp)
                            else:
                                nc.scalar.activation(out=g2(esc), in_=g2(s_ps),
                                                     func=Exp)
                            for h in hooks.get(kt, ()):
                                h()
                            if (kt == 14 and nxt is not None
                                    and "nopre" not in ablate):
                                pre_cell[0] = score_mms(nxt[0], nxt[1], nxt[2], 0)
                            if prev is not None:
                                attnv(*prev)
                            prev = (esc, kt)
                        attnv(*prev)

                        # normalize: att = at / den ; den in row 64 (ones col)
                        den = dnp.tile([1, 1024], f32, tag="den")
                        if "dvecopy" in ablate:
                            nc.vector.tensor_copy(den[0:1, 0:w], at_a[64:65, 0:w])
                            nc.vector.tensor_copy(den[0:1, 512:512 + w],
                                                  at_b[64:65, 0:w])
                        else:
                            nc.scalar.activation(out=den[0:1, 0:w],
                                                 in_=at_a[64:65, 0:w], func=Ident)
                            nc.scalar.activation(out=den[0:1, 512:512 + w],
                                                 in_=at_b[64:65, 0:w], func=Ident)
                        r2 = dnp.tile([128, 1024], f32, tag="r2")
                        nc.gpsimd.partition_broadcast(g2(r2), g2(den))
                        r2r = dnp.tile([128, 1024], f32, tag="r2r")
                        nc.vector.reciprocal_approx_fast(out=g2(r2r), in_=g2(r2))
                        nc.vector.tensor_mul(
                            att_sb[0:64, p, c0:c0 + w], at_a[0:64, 0:w],
                            r2r[0:64, 0:w])
                        nc.vector.tensor_mul(
                            att_sb[64:128, p, c0:c0 + w], at_b[0:64, 0:w],
                            r2r[64:128, 512:512 + w])

                    def out_ln_a(tt):
                        """Out-proj matmuls + residual add + LN stats for tile tt."""
                        o_full = sc.tile([128, 1024], f32, tag="s")
                        o_ps = o_full[:, 0:512]
                        for k in range(4):
                            nc.tensor.matmul(
                                o_ps, att_sb[:, k, 128 * tt:128 * (tt + 1)],
                                wo_sb[:, k, :], start=(k == 0), stop=(k == 3))
                        nc.vector.tensor_add(xstash[:, tt, :], o_ps,
                                             xq_res_sb[:, tt, :])
                        stats = ln.tile([128, 6], f32, tag="st")
                        nc.vector.bn_stats(stats, xstash[:, tt, :])
                        nc.vector.bn_aggr(mvst[:, tt, :], stats)

                    def rsqrt_batch(g):
                        """rstd = 1/sqrt(var+eps) for tiles 4g..4g+3 on DVE only
                        (magic-constant seed + 2 Newton steps; no ACT table)."""
                        sl = slice(4 * g, 4 * g + 4)
                        i32 = mybir.dt.int32
                        v = ln.tile([128, 4], f32, tag="rv")
                        nc.vector.tensor_scalar_add(v, mvst[:, sl, 1], LN_EPS)
                        hh = ln.tile([128, 4], f32, tag="rh")
                        nc.vector.tensor_scalar_mul(hh, v, -0.5)
                        y0 = ln.tile([128, 4], f32, tag="ry")
                        nc.vector.tensor_scalar(
                            out=y0.bitcast(i32), in0=v.bitcast(i32),
                            scalar1=1, scalar2=-1,
                            op0=Alu.arith_shift_right, op1=Alu.bitwise_xor)
                        y1 = ln.tile([128, 4], f32, tag="ry1")
                        nc.vector.tensor_scalar_add(
                            y1.bitcast(i32), y0.bitcast(i32), 0x5f375a87)
                        yy = y1
                        for it in range(2):
                            a = ln.tile([128, 4], f32, tag=f"ra{it}")
                            nc.vector.tensor_mul(a, yy, yy)
                            nc.vector.tensor_mul(a, a, hh)
                            nc.vector.tensor_scalar_add(a, a, 1.5)
                            dst = rstdst[:, sl] if it == 1 else ln.tile(
                                [128, 4], f32, tag=f"rn{it}")
                            nc.vector.tensor_mul(dst, yy, a)
                            yy = dst
                        nc.vector.scalar_tensor_tensor(
                            out=nmrst[:, sl], in0=mvst[:, sl, 0], scalar=-1.0,
                            in1=rstdst[:, sl], op0=Alu.mult, op1=Alu.mult)

                    def out_ln_b(tt):
                        """y = (x - mu) * rstd * gamma + beta, then store."""
                        y = ln.tile([128, E], f32, tag="y")
                        da = ln.tile([128, 1], f32, tag="da")
                        nc.vector.affine_mul_reduce(
                            out=y, accum_out=da, in0=xstash[:, tt, :],
                            in1=gamma_b, scale=rstdst[:, tt:tt + 1],
                            bias=nmrst[:, tt:tt + 1])
                        y2 = ln.tile([128, E], f32, tag="y2")
                        eng = nc.gpsimd if ln_eng == "gpsimd" else nc.vector
                        eng.tensor_add(y2, y, beta_b)
                        nc.sync.dma_start(out_d[128 * tt:128 * (tt + 1), :], y2)

                    # ---- main loop: pairs x query-halves; the final half is
                    # split 384/128 so softmax-normalize + out_ln overlap ----
                    WA = 384
                    for p in range(NPAIR):
                        for n in range(2):
                            if p == NPAIR - 1 and n == 1:
                                break
                            hooks = {}
                            if p == 0 and n == 0:
                                # just-in-time V projection + rest of K(0)
                                for kt in range(KT):
                                    hooks[kt] = [(lambda t=kt: proj_v(t))]
                                hooks[2].append(lambda: proj_kh(0, 1))
                                hooks[6].append(lambda: proj_k(0, 1))
                            if n == 1 and p < NPAIR - 1:
                                hooks[8] = [lambda q=p + 1: proj_q(q),
                                            lambda q=p + 1: proj_k(q, 0)]
                            if n == 0 and p > 0:
                                hooks[4] = [lambda q=p: proj_k(q, 1)]
                            if p == NPAIR - 1 and n == 0:
                                nxt = (p, 512, WA)
                            elif n == 0:
                                nxt = (p, 512, 512)
                            else:
                                nxt = (p + 1, 0, 512)
                            attention(p, 512 * n, 512, hooks, nxt=nxt)
                    if "nosplit" in ablate:
                        hooksA = {}
                        for i in range(4):
                            hooksA.setdefault(7 + 2 * i, []).append(
                                lambda t=i: out_ln_a(t))
                        hooksA.setdefault(15, []).append(lambda: rsqrt_batch(0))
                        attention(NPAIR - 1, 512, 512, hooksA)
                        for tt in range(4):
                            out_ln_b(tt)
                        for tt in range(4, TT):
                            out_ln_a(tt)
                        rsqrt_batch(1)
                        for tt in range(4, TT):
                            out_ln_b(tt)
                    else:
                        hooksA = {}
                        for i in range(4):
                            # (3,0) normalize drains ~5us into quarter A
                            hooksA.setdefault(7 + 2 * i, []).append(
                                lambda t=i: out_ln_a(t))
                        attention(NPAIR - 1, 512, WA, hooksA,
                                  nxt=(NPAIR - 1, 512 + WA, 512 - WA))
                        hooksB = {0: [lambda: rsqrt_batch(0)],
                                  2: [lambda: out_ln_a(4)],
                                  4: [lambda: out_ln_b(0)],
                                  6: [lambda: out_ln_a(5)],
                                  8: [lambda: out_ln_b(1)],
                                  10: [lambda: out_ln_b(2)],
                                  12: [lambda: out_ln_a(6)],
                                  14: [lambda: out_ln_b(3)]}
                        attention(NPAIR - 1, 512 + WA, 512 - WA, hooksB)
                        out_ln_a(7)
                        rsqrt_batch(1)
                        for tt in range(4, TT):
                            out_ln_b(tt)

            if reps == 1:
                body()
            else:
                with tc.For_i(0, reps, 1):
                    body()

    nc.compile()
    return nc


def shard_inputs(query_modality, key_modality, value_modality, Wq, bq, Wk, bk,
                 Wv, bv, Wo, bo, modal_compat, ln_gamma, ln_beta):
    """Host-side prep: fold compat into Wq/bq, bv/bo into the residual,
    pre-transpose, cast matmul operands to bf16, shard per core."""
    import ml_dtypes
    f32 = np.float32
    bf = ml_dtypes.bfloat16
    q = np.asarray(query_modality, f32)
    k = np.asarray(key_modality, f32)
    v = np.asarray(value_modality, f32)
    compat = np.asarray(modal_compat, f32).reshape(H)
    scale = np.repeat(compat / np.sqrt(HD), HD).astype(f32)     # [E]
    wq_eff = (np.asarray(Wq, f32) * scale[None, :]).astype(f32)
    bq_eff = (np.asarray(bq, f32) * scale).astype(f32)
    # attn rows sum to 1 => (attended + bv) @ Wo + bo = attended @ Wo + fold
    res_fold = (np.asarray(bv, f32) @ np.asarray(Wo, f32)
                + np.asarray(bo, f32)).astype(f32)              # [E]

    shared = {
        "wq": wq_eff.astype(bf), "wk": np.asarray(Wk, f32).astype(bf),
        "wv": np.asarray(Wv, f32).astype(bf),
        "wo": np.asarray(Wo, f32).astype(bf),
        "bq": bq_eff, "bk": np.asarray(bk, f32),
        "gamma": np.asarray(ln_gamma, f32), "beta": np.asarray(ln_beta, f32),
    }
    kt_cache = {}
    in_maps = []
    for c in range(N_CORES):
        b, half = c // 2, c % 2
        sl = slice(half * T, (half + 1) * T)
        if b not in kt_cache:
            kt_cache[b] = (np.ascontiguousarray(k[b].T).astype(bf),
                           np.ascontiguousarray(v[b].T).astype(bf))
        m = dict(shared)
        m["xqt"] = np.ascontiguousarray(q[b, sl, :].T).astype(bf)
        m["xkt"], m["xvt"] = kt_cache[b]
        m["xq_res"] = np.ascontiguousarray(q[b, sl, :] + res_fold[None, :])
        in_maps.append(m)
    return in_maps


def kernel(**inputs) -> np.ndarray:
    from concourse.bass_utils import run_bass_kernel_spmd

    if "nc" not in _CACHE:
        _CACHE["nc"] = build_nc(reps=1, **BUILD_KW)
    nc = _CACHE["nc"]
    in_maps = shard_inputs(**inputs)
    res = run_bass_kernel_spmd(nc, in_maps, core_ids=list(range(N_CORES)))
    out = np.empty((B, S, E), np.float32)
    for c in range(N_CORES):
        b, half = c // 2, c % 2
        out[b, half * T:(half + 1) * T, :] = res.results[c]["out"]
    return out



# revision 46
# speedup vs baseline: 1.0661x; 1.0148x over previous
"""CrossModalAttention Trainium2 kernel v2 (8 NeuronCores, SPMD, no collectives).

Reference computation (B=4, S=2048, E=512, H=8, HD=64):
  Q = q_mod @ Wq + bq ; K = k_mod @ Wk + bk ; V = v_mod @ Wv + bv   (per head)
  scores = (Q K^T / sqrt(HD)) * modal_compat[h] ; attn = softmax(scores)
  out = (attn @ V) @ Wo + bo ; LayerNorm(out + q_mod) * gamma + beta

Sharding: core c handles batch b=c//2, query-rows half=c%2 (1024 rows each).
K/V are computed per batch on both cores of a pair (duplicated, no collectives).

v2 changes vs v1:
  - bf16 matmul operands (2 cols/cycle rhs streaming; fp32 PSUM accum).
  - score matmuls row-tiled: the two heads of a pair contract over disjoint
    PE row groups (partitions 0-63 / 64-127) and run concurrently.
  - attention processed per (head-pair, query-half): score PSUM tiles
    [128,1024] double-buffered (4 banks) + attended [65,512] x2 (2 banks)
    + V/out-proj pool (2 banks) = 8 banks, so score matmuls of tile t+1
    overlap the exp of tile t (ScalarE runs at ~100% duty).
  - attnV software-pipelined one kt behind exp.
  - V projection emitted just-in-time per key-tile inside the pair-0 loop.
  - softmax denominators: GPSIMD partition_broadcast + DVE
    reciprocal_approx_fast (replaces DRAM round-trip + 8-cyc/elem divide).
  - bv/bo folded on host into the residual (attn rows sum to 1 =>
    attended@Wo + bv@Wo + bo absorbed into xq_res); modal_compat/sqrt(HD)
    folded into Wq/bq as before.
  - LayerNorm uses fused scalar_tensor_tensor ops; the beta/gamma stage
    runs on GPSIMD to shorten the DVE tail.
"""
import sys
sys.path.insert(0, "/opt/trn_rl_repo")
import numpy as np

B, S, E, H = 4, 2048, 512, 8
HD = E // H
LN_EPS = 1e-5
N_CORES = 8
T = S // 2          # query rows per core
KT = S // 128       # key tiles (16)
TT = T // 128       # out row tiles per core (8)
NPAIR = H // 2      # head pairs (4)

_CACHE = {}


class _null_ctx:
    def __enter__(self):
        return None

    def __exit__(self, *a):
        return False


BUILD_KW = dict(ln_eng="vector", ablate=("nopre", "dvecopy"))


def build_nc(reps: int = 1, mmdt: str = "bf16", bcast: str = "gpsimd",
             ln_eng: str = "vector", recip: str = "fast", ablate: tuple = ()):
    import concourse.tile as tile
    from concourse import bacc, mybir
    import concourse.bass as bass

    f32 = mybir.dt.float32
    f32r = mybir.dt.float32r
    bf16 = mybir.dt.bfloat16
    mdt = {"bf16": bf16, "f32r": f32r}[mmdt]
    Exp = mybir.ActivationFunctionType.Exp
    Ident = mybir.ActivationFunctionType.Identity
    Alu = mybir.AluOpType

    def bias_copy(out, in_, bias):
        """PSUM->SBUF add-bias copy: on ACT (idle on HW) unless 'dvecopy'."""
        if "dvecopy" in ablate:
            nc.vector.tensor_scalar_add(out, in_, scalar1=bias)
        else:
            nc.scalar.activation(out=out, in_=in_, func=Ident, bias=bias)

    nc = bacc.Bacc("TRN2", target_bir_lowering=False, debug=False,
                   enable_asserts=True, num_devices=N_CORES)
    dram = {}
    for name, shape, dt in [
        ("xqt", (E, T), mdt), ("xkt", (E, S), mdt), ("xvt", (E, S), mdt),
        ("wq", (E, E), mdt), ("wk", (E, E), mdt), ("wv", (E, E), mdt),
        ("wo", (E, E), mdt),
        ("bq", (E,), f32), ("bk", (E,), f32),
        ("xq_res", (T, E), f32), ("gamma", (E,), f32), ("beta", (E,), f32),
    ]:
        dram[name] = nc.dram_tensor(name, shape, dt, kind="ExternalInput").ap()
    out_d = nc.dram_tensor("out", (T, E), f32, kind="ExternalOutput").ap()

    def pbcast(ap, parts):
        """AP view broadcasting partition dim (step 0) to `parts`."""
        return bass.AP(tensor=ap.tensor, offset=ap.offset,
                       ap=[[0, parts]] + list(ap.ap[1:]))

    with tile.TileContext(nc) as tc:
        with tc.tile_pool(name="consts", bufs=1) as consts, \
             tc.tile_pool(name="persist", bufs=1) as persist:
            # weights / biases / constants
            wq_sb = consts.tile([128, 4, E], mdt)
            wk_sb = consts.tile([128, 4, E], mdt)
            wv_sb = consts.tile([128, 4, E], mdt)
            wo_sb = consts.tile([128, 4, E], mdt)
            bq_sb = consts.tile([128, 4], f32)
            bk_sb = consts.tile([128, 4], f32)
            gamma_b = consts.tile([128, E], f32)
            beta_b = consts.tile([128, E], f32)
            eps_sb = consts.tile([128, 1], f32)

            # persistent activations
            if "smallexp" in ablate:
                esc_dummy = persist.tile([128, 1024], mdt)
                if mmdt == "bf16":
                    # bf16 1.0 == 0x3F80 == 16256
                    nc.gpsimd.memset(esc_dummy[:].bitcast(mybir.dt.int16), 16256)
                else:
                    nc.gpsimd.memset(esc_dummy[:].bitcast(mybir.dt.float32), 1.0)
            # input activations, split into halves so DMA completion deps
            # stay per-half (whole-tile tracking merges waits across writers)
            xqt_h = [persist.tile([128, 4, T // 2], mdt, name=f"xqt{i}")
                     for i in range(2)]
            xkt_h = [persist.tile([128, 4, S // 2], mdt, name=f"xkt{i}")
                     for i in range(2)]
            xvt_h = [persist.tile([128, 4, S // 2], mdt, name=f"xvt{i}")
                     for i in range(2)]
            qt_sb = persist.tile([128, 4, T], mdt)     # Q.T feature-major
            kt_sb = persist.tile([128, 4, S], mdt)     # K.T feature-major
            v_sb = persist.tile([128, KT, H, HD + 1], mdt)  # V tokens + ones
            att_sb = persist.tile([128, 4, T], mdt)    # attended.T normalized
            xq_res_sb = persist.tile([128, TT, E], f32)
            xstash = persist.tile([128, TT, E], f32)   # out-proj + residual
            mvst = persist.tile([128, TT, 2], f32)     # LN (mu, var) per tile
            rstdst = persist.tile([128, TT], f32)      # LN rstd per tile
            nmrst = persist.tile([128, TT], f32)       # LN -mu*rstd per tile

            nc.gpsimd.memset(eps_sb, LN_EPS)
            # ones column of V (col HD of each head group): bf16 1.0 == 0x3F80
            if mmdt == "bf16":
                nc.gpsimd.memset(
                    v_sb[:, :, :, HD:HD + 1].bitcast(mybir.dt.int16), 16256)
            else:
                nc.gpsimd.memset(v_sb[:, :, :, HD:HD + 1], 1.0)

            def body():
                sc3 = "sc2bufs" not in ablate
                with tc.tile_pool(name="sc", bufs=3 if sc3 else 2,
                                  space="PSUM") as sc, \
                     tc.tile_pool(name="atp", bufs=1, space="PSUM") as atp, \
                     _null_ctx() if sc3 else tc.tile_pool(
                         name="vp", bufs=2, space="PSUM") as vp, \
                     tc.tile_pool(name="escp", bufs=4 if sc3 else 3) as escp, \
                     tc.tile_pool(name="dnp", bufs=2) as dnp, \
                     tc.tile_pool(name="dndp", bufs=2, space="DRAM") as dndp, \
                     tc.tile_pool(name="ln", bufs=2) as ln:

                    # ---- input DMAs (ordered by first use on the critical
                    # path: Q-proj needs wq+xqt, first scores need wk+xkt h0)
                    xq_r = dram["xqt"].rearrange("(k p) t -> p k t", p=128)
                    xk_r = dram["xkt"].rearrange("(k p) t -> p k t", p=128)
                    xv_r = dram["xvt"].rearrange("(k p) t -> p k t", p=128)
                    # Q-path inputs ride the ACT HWDGE queue so they overlap
                    # the K-path loads on the SP queue (done well before the
                    # first exp needs the ACT engine itself)
                    qeng = nc.sync if "spdma" in ablate else nc.scalar
                    qeng.dma_start(xqt_h[0], xq_r[:, :, 0:512])
                    qeng.dma_start(
                        wq_sb, dram["wq"].rearrange("(k p) e -> p k e", p=128))
                    qeng.dma_start(xqt_h[1], xq_r[:, :, 512:1024])
                    nc.sync.dma_start(
                        wk_sb, dram["wk"].rearrange("(k p) e -> p k e", p=128))
                    nc.sync.dma_start(bk_sb, dram["bk"].rearrange("(m p) -> p m", p=128))
                    nc.sync.dma_start(bq_sb, dram["bq"].rearrange("(m p) -> p m", p=128))
                    nc.sync.dma_start(xkt_h[0], xk_r[:, :, 0:1024])
                    nc.sync.dma_start(
                        wv_sb, dram["wv"].rearrange("(k p) e -> p k e", p=128))
                    nc.sync.dma_start(xvt_h[0], xv_r[:, :, 0:1024])
                    nc.sync.dma_start(xkt_h[1], xk_r[:, :, 1024:2048])
                    nc.sync.dma_start(xvt_h[1], xv_r[:, :, 1024:2048])
                    nc.sync.dma_start(
                        wo_sb, dram["wo"].rearrange("(k p) e -> p k e", p=128))
                    nc.sync.dma_start(gamma_b, pbcast(dram["gamma"][None, :], 128))
                    nc.sync.dma_start(beta_b, pbcast(dram["beta"][None, :], 128))
                    nc.sync.dma_start(
                        xq_res_sb, dram["xq_res"].rearrange("(t p) e -> p t e", p=128))

                    def proj_q(p):
                        q_ps = sc.tile([128, 1024], f32, tag="s")
                        for nn in range(2):
                            for k in range(4):
                                nc.tensor.matmul(
                                    q_ps[:, 512 * nn:512 * (nn + 1)],
                                    wq_sb[:, k, 128 * p:128 * (p + 1)],
                                    xqt_h[nn][:, k, :],
                                    start=(k == 0), stop=(k == 3))
                        bias_copy(qt_sb[:, p, :], q_ps, bq_sb[:, p:p + 1])

                    def proj_kh(p, g4):
                        """K projection for pair p, 512-key group g4 (0..3)."""
                        k_ps = sc.tile([128, 1024], f32, tag="s")
                        c0, off = 512 * g4, 512 * (g4 % 2)
                        for k in range(4):
                            nc.tensor.matmul(
                                k_ps[:, 0:512],
                                wk_sb[:, k, 128 * p:128 * (p + 1)],
                                xkt_h[g4 // 2][:, k, off:off + 512],
                                start=(k == 0), stop=(k == 3))
                        bias_copy(kt_sb[:, p, c0:c0 + 512], k_ps[:, 0:512],
                                  bk_sb[:, p:p + 1])

                    def proj_k(p, g2):
                        proj_kh(p, 2 * g2)
                        proj_kh(p, 2 * g2 + 1)

                    def proj_v(tt):
                        if sc3:
                            v_full = sc.tile([128, 1024], f32, tag="s")
                            v_ps = v_full[:, 0:512]
                        else:
                            v_ps = vp.tile([128, 512], f32, tag="v")
                        off = 128 * (tt % 8)
                        for k in range(4):
                            nc.tensor.matmul(
                                v_ps, xvt_h[tt // 8][:, k, off:off + 128],
                                wv_sb[:, k, :], start=(k == 0), stop=(k == 3))
                        if "dvecopy" in ablate:
                            nc.vector.tensor_copy(
                                v_sb[:, tt, :, 0:HD],
                                v_ps.rearrange("p (h d) -> p h d", h=H))
                        else:
                            nc.scalar.activation(
                                out=v_sb[:, tt, :, 0:HD],
                                in_=v_ps.rearrange("p (h d) -> p h d", h=H),
                                func=Ident)

                    # ---- startup projections for pair 0 ----
                    proj_q(0)
                    proj_kh(0, 0)

                    pre_cell = [None]  # pre-issued kt0 score tile for next phase

                    def attention(p, c0, w, hooks, nxt=None):
                        """Attention for head pair p, query columns [c0, c0+w).
                        nxt=(p',c0',w'): pre-issue that phase's kt0 scores at
                        kt14 so its first exp follows ours with no ACT gap."""
                        at_a = atp.tile([65, 512], f32, tag="atA")
                        at_b = atp.tile([65, 512], f32, tag="atB")
                        prev = None

                        def g2(t):
                            # [128, 2, w] view: head A at col 0, head B at 512
                            # (PSUM-bank-aligned for any w <= 512)
                            return t.rearrange("p (g x) -> p g x", g=2)[:, :, 0:w]

                        def attnv(esc, kt):
                            if "fewattnv" in ablate and kt not in (0, KT - 1):
                                return  # timing probe: WRONG results
                            nc.tensor.matmul(
                                at_a[:, 0:w], v_sb[:, kt, 2 * p, :], esc[:, 0:w],
                                start=(kt == 0), stop=(kt == KT - 1))
                            nc.tensor.matmul(
                                at_b[:, 0:w], v_sb[:, kt, 2 * p + 1, :],
                                esc[:, 512:512 + w],
                                start=(kt == 0), stop=(kt == KT - 1))

                        def score_mms(pp, cc0, ww, kt):
                            s_ps = sc.tile([128, 1024], f32, tag="s")
                            if "score1mm" in ablate:  # timing probe: WRONG
                                nc.tensor.matmul(
                                    s_ps[:, 0:ww],
                                    kt_sb[:, pp, 128 * kt:128 * (kt + 1)],
                                    qt_sb[:, pp, cc0:cc0 + ww],
                                    start=True, stop=True)
                                nc.tensor.matmul(
                                    s_ps[:, 512:512 + 64],
                                    kt_sb[:, pp, 128 * kt:128 * (kt + 1)],
                                    qt_sb[:, pp, cc0:cc0 + 64],
                                    start=True, stop=True)
                                return s_ps
                            if "norowtile" in ablate:
                                nc.tensor.matmul(
                                    s_ps[:, 0:ww],
                                    kt_sb[0:64, pp, 128 * kt:128 * (kt + 1)],
                                    qt_sb[0:64, pp, cc0:cc0 + ww],
                                    start=True, stop=True)
                                nc.tensor.matmul(
                                    s_ps[:, 512:512 + ww],
                                    kt_sb[64:128, pp, 128 * kt:128 * (kt + 1)],
                                    qt_sb[64:128, pp, cc0:cc0 + ww],
                                    start=True, stop=True)
                                return s_ps
                            nc.tensor.matmul(
                                s_ps[:, 0:ww],
                                kt_sb[0:64, pp, 128 * kt:128 * (kt + 1)],
                                qt_sb[0:64, pp, cc0:cc0 + ww],
                                start=True, stop=True, tile_position=(0, 0))
                            nc.tensor.matmul(
                                s_ps[:, 512:512 + ww],
                                kt_sb[64:128, pp, 128 * kt:128 * (kt + 1)],
                                qt_sb[64:128, pp, cc0:cc0 + ww],
                                start=True, stop=True, tile_position=(64, 0))
                            return s_ps

                        for kt in range(KT):
                            if kt == 0 and pre_cell[0] is not None:
                                s_ps = pre_cell[0]
                                pre_cell[0] = None
                            else:
                                s_ps = score_mms(p, c0, w, kt)
                            esc = escp.tile([128, 1024], mdt, tag="esc")
                            if "tinyexp" in ablate:  # timing probe: WRONG
                                nc.scalar.activation(out=esc[:, 0:64],
                                                     in_=s_ps[:, 0:64], func=Exp)
                            else:
                                nc.scalar.activation(out=g2(esc), in_=g2(s_ps),
                                                     func=Exp)
                            for h in hooks.get(kt, ()):
                                h()
                            if (kt == 14 and nxt is not None
                                    and "nopre" not in ablate):
                                pre_cell[0] = score_mms(nxt[0], nxt[1], nxt[2], 0)
                            if prev is not None:
                                attnv(*prev)
                            prev = (esc, kt)
                        attnv(*prev)

                        # normalize: att = at / den ; den in row 64 (ones col)
                        den = dnp.tile([1, 1024], f32, tag="den")
                        if "dvecopy" in ablate:
                            nc.vector.tensor_copy(den[0:1, 0:w], at_a[64:65, 0:w])
                            nc.vector.tensor_copy(den[0:1, 512:512 + w],
                                                  at_b[64:65, 0:w])
                        else:
                            nc.scalar.activation(out=den[0:1, 0:w],
                                                 in_=at_a[64:65, 0:w], func=Ident)
                            nc.scalar.activation(out=den[0:1, 512:512 + w],
                                                 in_=at_b[64:65, 0:w], func=Ident)
                        r2 = dnp.tile([128, 1024], f32, tag="r2")
                        nc.gpsimd.partition_broadcast(g2(r2), g2(den))
                        r2r = dnp.tile([128, 1024], f32, tag="r2r")
                        nc.vector.reciprocal_approx_fast(out=g2(r2r), in_=g2(r2))
                        nc.vector.tensor_mul(
                            att_sb[0:64, p, c0:c0 + w], at_a[0:64, 0:w],
                            r2r[0:64, 0:w])
                        nc.vector.tensor_mul(
                            att_sb[64:128, p, c0:c0 + w], at_b[0:64, 0:w],
                            r2r[64:128, 512:512 + w])

                    def out_ln_a(tt):
                        """Out-proj matmuls + residual add + LN stats for tile tt."""
                        o_full = sc.tile([128, 1024], f32, tag="s")
                        o_ps = o_full[:, 0:512]
                        for k in range(4):
                            nc.tensor.matmul(
                                o_ps, att_sb[:, k, 128 * tt:128 * (tt + 1)],
                                wo_sb[:, k, :], start=(k == 0), stop=(k == 3))
                        nc.vector.tensor_add(xstash[:, tt, :], o_ps,
                                             xq_res_sb[:, tt, :])
                        stats = ln.tile([128, 6], f32, tag="st")
                        nc.vector.bn_stats(stats, xstash[:, tt, :])
                        nc.vector.bn_aggr(mvst[:, tt, :], stats)

                    def rsqrt_batch(g):
                        """rstd = 1/sqrt(var+eps) for tiles 4g..4g+3 on DVE only
                        (magic-constant seed + 2 Newton steps; no ACT table)."""
                        sl = slice(4 * g, 4 * g + 4)
                        i32 = mybir.dt.int32
                        v = ln.tile([128, 4], f32, tag="rv")
                        nc.vector.tensor_scalar_add(v, mvst[:, sl, 1], LN_EPS)
                        hh = ln.tile([128, 4], f32, tag="rh")
                        nc.vector.tensor_scalar_mul(hh, v, -0.5)
                        y0 = ln.tile([128, 4], f32, tag="ry")
                        nc.vector.tensor_scalar(
                            out=y0.bitcast(i32), in0=v.bitcast(i32),
                            scalar1=1, scalar2=-1,
                            op0=Alu.arith_shift_right, op1=Alu.bitwise_xor)
                        y1 = ln.tile([128, 4], f32, tag="ry1")
                        nc.vector.tensor_scalar_add(
                            y1.bitcast(i32), y0.bitcast(i32), 0x5f375a87)
                        yy = y1
                        for it in range(2):
                            a = ln.tile([128, 4], f32, tag=f"ra{it}")
                            nc.vector.tensor_mul(a, yy, yy)
                            nc.vector.tensor_mul(a, a, hh)
                            nc.vector.tensor_scalar_add(a, a, 1.5)
                            dst = rstdst[:, sl] if it == 1 else ln.tile(
                                [128, 4], f32, tag=f"rn{it}")
                            nc.vector.tensor_mul(dst, yy, a)
                            yy = dst
                        nc.vector.scalar_tensor_tensor(
                            out=nmrst[:, sl], in0=mvst[:, sl, 0], scalar=-1.0,
                            in1=rstdst[:, sl], op0=Alu.mult, op1=Alu.mult)

                    def out_ln_b(tt):
                        """y = (x - mu) * rstd * gamma + beta, then store."""
                        y = ln.tile([128, E], f32, tag="y")
                        da = ln.tile([128, 1], f32, tag="da")
                        nc.vector.affine_mul_reduce(
                            out=y, accum_out=da, in0=xstash[:, tt, :],
                            in1=gamma_b, scale=rstdst[:, tt:tt + 1],
                            bias=nmrst[:, tt:tt + 1])
                        y2 = ln.tile([128, E], f32, tag="y2")
                        eng = nc.gpsimd if ln_eng == "gpsimd" else nc.vector
                        eng.tensor_add(y2, y, beta_b)
                        nc.sync.dma_start(out_d[128 * tt:128 * (tt + 1), :], y2)

                    # ---- main loop: pairs x query-halves; the final half is
                    # split 384/128 so softmax-normalize + out_ln overlap ----
                    WA = 320 if "wa320" in ablate else 384
                    for p in range(NPAIR):
                        for n in range(2):
                            if p == NPAIR - 1 and n == 1:
                                break
                            hooks = {}
                            if p == 0 and n == 0:
                                # just-in-time V projection + rest of K(0)
                                for kt in range(KT):
                                    hooks[kt] = [(lambda t=kt: proj_v(t))]
                                hooks[2].append(lambda: proj_kh(0, 1))
                                hooks[6].append(lambda: proj_k(0, 1))
                            if n == 1 and p < NPAIR - 1:
                                hooks[8] = [lambda q=p + 1: proj_q(q),
                                            lambda q=p + 1: proj_k(q, 0)]
                            if n == 0 and p > 0:
                                hooks[4] = [lambda q=p: proj_k(q, 1)]
                            if p == NPAIR - 1 and n == 0:
                                nxt = (p, 512, WA)
                            elif n == 0:
                                nxt = (p, 512, 512)
                            else:
                                nxt = (p + 1, 0, 512)
                            attention(p, 512 * n, 512, hooks, nxt=nxt)
                    if "nosplit" in ablate:
                        hooksA = {}
                        for i in range(4):
                            hooksA.setdefault(7 + 2 * i, []).append(
                                lambda t=i: out_ln_a(t))
                        hooksA.setdefault(15, []).append(lambda: rsqrt_batch(0))
                        attention(NPAIR - 1, 512, 512, hooksA)
                        for tt in range(4):
                            out_ln_b(tt)
                        for tt in range(4, TT):
                            out_ln_a(tt)
                        rsqrt_batch(1)
                        for tt in range(4, TT):
                            out_ln_b(tt)
                    else:
                        hooksA = {}
                        for i in range(4):
                            # (3,0) normalize drains ~5us into quarter A
                            hooksA.setdefault(7 + 2 * i, []).append(
                                lambda t=i: out_ln_a(t))
                        attention(NPAIR - 1, 512, WA, hooksA,
                                  nxt=(NPAIR - 1, 512 + WA, 512 - WA))
                        hooksB = {0: [lambda: rsqrt_batch(0)],
                                  2: [lambda: out_ln_a(4)],
                                  4: [lambda: out_ln_b(0)],
                                  6: [lambda: out_ln_a(5)],
                                  8: [lambda: out_ln_b(1)],
                                  10: [lambda: out_ln_b(2)],
                                  14: [lambda: out_ln_b(3)]}
                        if WA >= 384:  # tile 6 (q 768:896) inside quarter A
                            hooksB[12] = [lambda: out_ln_a(6)]
                        attention(NPAIR - 1, 512 + WA, 512 - WA, hooksB)
                        if WA < 384:
                            out_ln_a(6)
                        out_ln_a(7)
                        rsqrt_batch(1)
                        for tt in range(4, TT):
                            out_ln_b(tt)

            if reps == 1:
                body()
            else:
                with tc.For_i(0, reps, 1):
                    body()

    nc.compile()
    return nc


def shard_inputs(query_modality, key_modality, value_modality, Wq, bq, Wk, bk,
                 Wv, bv, Wo, bo, modal_compat, ln_gamma, ln_beta):
    """Host-side prep: fold compat into Wq/bq, bv/bo into the residual,
    pre-transpose, cast matmul operands to bf16, shard per core."""
    import ml_dtypes
    f32 = np.float32
    bf = ml_dtypes.bfloat16
    q = np.asarray(query_modality, f32)
    k = np.asarray(key_modality, f32)
    v = np.asarray(value_modality, f32)
    compat = np.asarray(modal_compat, f32).reshape(H)
    scale = np.repeat(compat / np.sqrt(HD), HD).astype(f32)     # [E]
    wq_eff = (np.asarray(Wq, f32) * scale[None, :]).astype(f32)
    bq_eff = (np.asarray(bq, f32) * scale).astype(f32)
    # attn rows sum to 1 => (attended + bv) @ Wo + bo = attended @ Wo + fold
    res_fold = (np.asarray(bv, f32) @ np.asarray(Wo, f32)
                + np.asarray(bo, f32)).astype(f32)              # [E]

    shared = {
        "wq": wq_eff.astype(bf), "wk": np.asarray(Wk, f32).astype(bf),
        "wv": np.asarray(Wv, f32).astype(bf),
        "wo": np.asarray(Wo, f32).astype(bf),
        "bq": bq_eff, "bk": np.asarray(bk, f32),
        "gamma": np.asarray(ln_gamma, f32), "beta": np.asarray(ln_beta, f32),
    }
    kt_cache = {}
    in_maps = []
    for c in range(N_CORES):
        b, half = c // 2, c % 2
        sl = slice(half * T, (half + 1) * T)
        if b not in kt_cache:
            kt_cache[b] = (np.ascontiguousarray(k[b].T).astype(bf),
                           np.ascontiguousarray(v[b].T).astype(bf))
        m = dict(shared)
        m["xqt"] = np.ascontiguousarray(q[b, sl, :].T).astype(bf)
        m["xkt"], m["xvt"] = kt_cache[b]
        m["xq_res"] = np.ascontiguousarray(q[b, sl, :] + res_fold[None, :])
        in_maps.append(m)
    return in_maps


def kernel(**inputs) -> np.ndarray:
    from concourse.bass_utils import run_bass_kernel_spmd

    if "nc" not in _CACHE:
        _CACHE["nc"] = build_nc(reps=1, **BUILD_KW)
    nc = _CACHE["nc"]
    in_maps = shard_inputs(**inputs)
    res = run_bass_kernel_spmd(nc, in_maps, core_ids=list(range(N_CORES)))
    out = np.empty((B, S, E), np.float32)
    for c in range(N_CORES):
        b, half = c // 2, c % 2
        out[b, half * T:(half + 1) * T, :] = res.results[c]["out"]
    return out



# revision 47
# speedup vs baseline: 1.0864x; 1.0190x over previous
"""CrossModalAttention Trainium2 kernel v2 (8 NeuronCores, SPMD, no collectives).

Reference computation (B=4, S=2048, E=512, H=8, HD=64):
  Q = q_mod @ Wq + bq ; K = k_mod @ Wk + bk ; V = v_mod @ Wv + bv   (per head)
  scores = (Q K^T / sqrt(HD)) * modal_compat[h] ; attn = softmax(scores)
  out = (attn @ V) @ Wo + bo ; LayerNorm(out + q_mod) * gamma + beta

Sharding: core c handles batch b=c//2, query-rows half=c%2 (1024 rows each).
K/V are computed per batch on both cores of a pair (duplicated, no collectives).

v2 changes vs v1:
  - bf16 matmul operands (2 cols/cycle rhs streaming; fp32 PSUM accum).
  - score matmuls row-tiled: the two heads of a pair contract over disjoint
    PE row groups (partitions 0-63 / 64-127) and run concurrently.
  - attention processed per (head-pair, query-half): score PSUM tiles
    [128,1024] double-buffered (4 banks) + attended [65,512] x2 (2 banks)
    + V/out-proj pool (2 banks) = 8 banks, so score matmuls of tile t+1
    overlap the exp of tile t (ScalarE runs at ~100% duty).
  - attnV software-pipelined one kt behind exp.
  - V projection emitted just-in-time per key-tile inside the pair-0 loop.
  - softmax denominators: GPSIMD partition_broadcast + DVE
    reciprocal_approx_fast (replaces DRAM round-trip + 8-cyc/elem divide).
  - bv/bo folded on host into the residual (attn rows sum to 1 =>
    attended@Wo + bv@Wo + bo absorbed into xq_res); modal_compat/sqrt(HD)
    folded into Wq/bq as before.
  - LayerNorm uses fused scalar_tensor_tensor ops; the beta/gamma stage
    runs on GPSIMD to shorten the DVE tail.
"""
import sys
sys.path.insert(0, "/opt/trn_rl_repo")
import numpy as np

B, S, E, H = 4, 2048, 512, 8
HD = E // H
LN_EPS = 1e-5
N_CORES = 8
T = S // 2          # query rows per core
KT = S // 128       # key tiles (16)
TT = T // 128       # out row tiles per core (8)
NPAIR = H // 2      # head pairs (4)

_CACHE = {}


class _null_ctx:
    def __enter__(self):
        return None

    def __exit__(self, *a):
        return False


BUILD_KW = dict(ln_eng="vector", ablate=("nopre", "dvecopy"))


def build_nc(reps: int = 1, mmdt: str = "bf16", bcast: str = "gpsimd",
             ln_eng: str = "vector", recip: str = "fast", ablate: tuple = ()):
    import concourse.tile as tile
    from concourse import bacc, mybir
    import concourse.bass as bass

    f32 = mybir.dt.float32
    f32r = mybir.dt.float32r
    bf16 = mybir.dt.bfloat16
    mdt = {"bf16": bf16, "f32r": f32r}[mmdt]
    Exp = mybir.ActivationFunctionType.Exp
    Ident = mybir.ActivationFunctionType.Identity
    Alu = mybir.AluOpType

    def bias_copy(out, in_, bias):
        """PSUM->SBUF add-bias copy: on ACT (idle on HW) unless 'dvecopy'."""
        if "dvecopy" in ablate:
            nc.vector.tensor_scalar_add(out, in_, scalar1=bias)
        else:
            nc.scalar.activation(out=out, in_=in_, func=Ident, bias=bias)

    nc = bacc.Bacc("TRN2", target_bir_lowering=False, debug=False,
                   enable_asserts=True, num_devices=N_CORES)
    dram = {}
    for name, shape, dt in [
        ("xqt", (E, T), mdt), ("xkt", (E, S), mdt), ("xvt", (E, S), mdt),
        ("wq", (E, E), mdt), ("wk", (E, E), mdt), ("wv", (E, E), mdt),
        ("wo", (E, E), mdt),
        ("bq", (E,), f32), ("bk", (E,), f32),
        ("xq_res", (T, E), f32), ("gamma", (E,), f32), ("beta", (E,), f32),
    ]:
        dram[name] = nc.dram_tensor(name, shape, dt, kind="ExternalInput").ap()
    out_d = nc.dram_tensor("out", (T, E), f32, kind="ExternalOutput").ap()

    def pbcast(ap, parts):
        """AP view broadcasting partition dim (step 0) to `parts`."""
        return bass.AP(tensor=ap.tensor, offset=ap.offset,
                       ap=[[0, parts]] + list(ap.ap[1:]))

    with tile.TileContext(nc) as tc:
        with tc.tile_pool(name="consts", bufs=1) as consts, \
             tc.tile_pool(name="persist", bufs=1) as persist:
            # weights / biases / constants
            wq_sb = consts.tile([128, 4, E], mdt)
            wk_sb = consts.tile([128, 4, E], mdt)
            wv_sb = consts.tile([128, 4, E], mdt)
            wo_sb = consts.tile([128, 4, E], mdt)
            bq_sb = consts.tile([128, 4], f32)
            bk_sb = consts.tile([128, 4], f32)
            gamma_b = consts.tile([128, E], f32)
            beta_b = consts.tile([128, E], f32)
            eps_sb = consts.tile([128, 1], f32)

            # persistent activations
            if "smallexp" in ablate:
                esc_dummy = persist.tile([128, 1024], mdt)
                if mmdt == "bf16":
                    # bf16 1.0 == 0x3F80 == 16256
                    nc.gpsimd.memset(esc_dummy[:].bitcast(mybir.dt.int16), 16256)
                else:
                    nc.gpsimd.memset(esc_dummy[:].bitcast(mybir.dt.float32), 1.0)
            # input activations, split into halves so DMA completion deps
            # stay per-half (whole-tile tracking merges waits across writers)
            xqt_h = [persist.tile([128, 4, T // 2], mdt, name=f"xqt{i}")
                     for i in range(2)]
            xkt_h = [persist.tile([128, 4, S // 2], mdt, name=f"xkt{i}")
                     for i in range(2)]
            xvt_h = [persist.tile([128, 4, S // 2], mdt, name=f"xvt{i}")
                     for i in range(2)]
            qt_sb = persist.tile([128, 4, T], mdt)     # Q.T feature-major
            kt_sb = persist.tile([128, 4, S], mdt)     # K.T feature-major
            v_sb = persist.tile([128, KT, H, HD + 1], mdt)  # V tokens + ones
            att_sb = persist.tile([128, 4, T], mdt)    # attended.T normalized
            xq_res_sb = persist.tile([128, TT, E], f32)
            xstash = persist.tile([128, TT, E], f32)   # out-proj + residual
            mvst = persist.tile([128, TT, 2], f32)     # LN (mu, var) per tile
            rstdst = persist.tile([128, TT], f32)      # LN rstd per tile
            nmrst = persist.tile([128, TT], f32)       # LN -mu*rstd per tile

            nc.gpsimd.memset(eps_sb, LN_EPS)
            # ones column of V (col HD of each head group): bf16 1.0 == 0x3F80
            if mmdt == "bf16":
                nc.gpsimd.memset(
                    v_sb[:, :, :, HD:HD + 1].bitcast(mybir.dt.int16), 16256)
            else:
                nc.gpsimd.memset(v_sb[:, :, :, HD:HD + 1], 1.0)

            def body():
                sc3 = "sc2bufs" not in ablate
                if "atp2" in ablate:
                    # double-buffer attnv accumulators so a phase's first attnv
                    # doesn't wait on the previous phase's normalize; costs one
                    # score buffer (PSUM: sc 2x2 + atp 2x2x1 = 8 banks)
                    sc_bufs, atp_bufs = 2, 2
                else:
                    sc_bufs, atp_bufs = (3 if sc3 else 2), 1
                with tc.tile_pool(name="sc", bufs=sc_bufs,
                                  space="PSUM") as sc, \
                     tc.tile_pool(name="atp", bufs=atp_bufs,
                                  space="PSUM") as atp, \
                     _null_ctx() if (sc3 or "atp2" in ablate) else tc.tile_pool(
                         name="vp", bufs=2, space="PSUM") as vp, \
                     tc.tile_pool(name="escp",
                                  bufs=6 if "escp6" in ablate
                                  else (4 if sc3 else 3)) as escp, \
                     tc.tile_pool(name="dnp", bufs=2) as dnp, \
                     tc.tile_pool(name="dndp", bufs=2, space="DRAM") as dndp, \
                     tc.tile_pool(name="ln",
                                  bufs=3 if "ln3" in ablate else 2) as ln:

                    # ---- input DMAs (ordered by first use on the critical
                    # path: Q-proj needs wq+xqt, first scores need wk+xkt h0)
                    xq_r = dram["xqt"].rearrange("(k p) t -> p k t", p=128)
                    xk_r = dram["xkt"].rearrange("(k p) t -> p k t", p=128)
                    xv_r = dram["xvt"].rearrange("(k p) t -> p k t", p=128)
                    # Q-path inputs ride the ACT HWDGE queue so they overlap
                    # the K-path loads on the SP queue (done well before the
                    # first exp needs the ACT engine itself)
                    qeng = nc.sync if "spdma" in ablate else nc.scalar
                    qeng.dma_start(xqt_h[0], xq_r[:, :, 0:512])
                    qeng.dma_start(
                        wq_sb, dram["wq"].rearrange("(k p) e -> p k e", p=128))
                    qeng.dma_start(xqt_h[1], xq_r[:, :, 512:1024])
                    nc.sync.dma_start(
                        wk_sb, dram["wk"].rearrange("(k p) e -> p k e", p=128))
                    nc.sync.dma_start(bk_sb, dram["bk"].rearrange("(m p) -> p m", p=128))
                    nc.sync.dma_start(bq_sb, dram["bq"].rearrange("(m p) -> p m", p=128))
                    nc.sync.dma_start(xkt_h[0], xk_r[:, :, 0:1024])
                    nc.sync.dma_start(
                        wv_sb, dram["wv"].rearrange("(k p) e -> p k e", p=128))
                    nc.sync.dma_start(xvt_h[0], xv_r[:, :, 0:1024])
                    nc.sync.dma_start(xkt_h[1], xk_r[:, :, 1024:2048])
                    nc.sync.dma_start(xvt_h[1], xv_r[:, :, 1024:2048])
                    nc.sync.dma_start(
                        wo_sb, dram["wo"].rearrange("(k p) e -> p k e", p=128))
                    nc.sync.dma_start(gamma_b, pbcast(dram["gamma"][None, :], 128))
                    nc.sync.dma_start(beta_b, pbcast(dram["beta"][None, :], 128))
                    nc.sync.dma_start(
                        xq_res_sb, dram["xq_res"].rearrange("(t p) e -> p t e", p=128))

                    def proj_q(p):
                        q_ps = sc.tile([128, 1024], f32, tag="s")
                        for nn in range(2):
                            for k in range(4):
                                nc.tensor.matmul(
                                    q_ps[:, 512 * nn:512 * (nn + 1)],
                                    wq_sb[:, k, 128 * p:128 * (p + 1)],
                                    xqt_h[nn][:, k, :],
                                    start=(k == 0), stop=(k == 3))
                        bias_copy(qt_sb[:, p, :], q_ps, bq_sb[:, p:p + 1])

                    def proj_kh(p, g4):
                        """K projection for pair p, 512-key group g4 (0..3)."""
                        k_ps = sc.tile([128, 1024], f32, tag="s")
                        c0, off = 512 * g4, 512 * (g4 % 2)
                        for k in range(4):
                            nc.tensor.matmul(
                                k_ps[:, 0:512],
                                wk_sb[:, k, 128 * p:128 * (p + 1)],
                                xkt_h[g4 // 2][:, k, off:off + 512],
                                start=(k == 0), stop=(k == 3))
                        bias_copy(kt_sb[:, p, c0:c0 + 512], k_ps[:, 0:512],
                                  bk_sb[:, p:p + 1])

                    def proj_k(p, g2):
                        proj_kh(p, 2 * g2)
                        proj_kh(p, 2 * g2 + 1)

                    def proj_v(tt):
                        if sc3:
                            v_full = sc.tile([128, 1024], f32, tag="s")
                            v_ps = v_full[:, 0:512]
                        else:
                            v_ps = vp.tile([128, 512], f32, tag="v")
                        off = 128 * (tt % 8)
                        for k in range(4):
                            nc.tensor.matmul(
                                v_ps, xvt_h[tt // 8][:, k, off:off + 128],
                                wv_sb[:, k, :], start=(k == 0), stop=(k == 3))
                        if "dvecopy" in ablate:
                            nc.vector.tensor_copy(
                                v_sb[:, tt, :, 0:HD],
                                v_ps.rearrange("p (h d) -> p h d", h=H))
                        else:
                            nc.scalar.activation(
                                out=v_sb[:, tt, :, 0:HD],
                                in_=v_ps.rearrange("p (h d) -> p h d", h=H),
                                func=Ident)

                    # ---- startup projections for pair 0 ----
                    proj_q(0)
                    proj_kh(0, 0)

                    pre_cell = [None]  # pre-issued kt0 score tile for next phase

                    def attention(p, c0, w, hooks, nxt=None):
                        """Attention for head pair p, query columns [c0, c0+w).
                        nxt=(p',c0',w'): pre-issue that phase's kt0 scores at
                        kt14 so its first exp follows ours with no ACT gap."""
                        at_a = atp.tile([65, 512], f32, tag="atA")
                        at_b = atp.tile([65, 512], f32, tag="atB")
                        prev = None

                        def g2(t):
                            # [128, 2, w] view: head A at col 0, head B at 512
                            # (PSUM-bank-aligned for any w <= 512)
                            return t.rearrange("p (g x) -> p g x", g=2)[:, :, 0:w]

                        def attnv(esc, kt):
                            if "fewattnv" in ablate and kt not in (0, KT - 1):
                                return  # timing probe: WRONG results
                            nc.tensor.matmul(
                                at_a[:, 0:w], v_sb[:, kt, 2 * p, :], esc[:, 0:w],
                                start=(kt == 0), stop=(kt == KT - 1))
                            nc.tensor.matmul(
                                at_b[:, 0:w], v_sb[:, kt, 2 * p + 1, :],
                                esc[:, 512:512 + w],
                                start=(kt == 0), stop=(kt == KT - 1))

                        def score_mms(pp, cc0, ww, kt):
                            s_ps = sc.tile([128, 1024], f32, tag="s")
                            if "score1mm" in ablate:  # timing probe: WRONG
                                nc.tensor.matmul(
                                    s_ps[:, 0:ww],
                                    kt_sb[:, pp, 128 * kt:128 * (kt + 1)],
                                    qt_sb[:, pp, cc0:cc0 + ww],
                                    start=True, stop=True)
                                nc.tensor.matmul(
                                    s_ps[:, 512:512 + 64],
                                    kt_sb[:, pp, 128 * kt:128 * (kt + 1)],
                                    qt_sb[:, pp, cc0:cc0 + 64],
                                    start=True, stop=True)
                                return s_ps
                            if "norowtile" in ablate:
                                nc.tensor.matmul(
                                    s_ps[:, 0:ww],
                                    kt_sb[0:64, pp, 128 * kt:128 * (kt + 1)],
                                    qt_sb[0:64, pp, cc0:cc0 + ww],
                                    start=True, stop=True)
                                nc.tensor.matmul(
                                    s_ps[:, 512:512 + ww],
                                    kt_sb[64:128, pp, 128 * kt:128 * (kt + 1)],
                                    qt_sb[64:128, pp, cc0:cc0 + ww],
                                    start=True, stop=True)
                                return s_ps
                            nc.tensor.matmul(
                                s_ps[:, 0:ww],
                                kt_sb[0:64, pp, 128 * kt:128 * (kt + 1)],
                                qt_sb[0:64, pp, cc0:cc0 + ww],
                                start=True, stop=True, tile_position=(0, 0))
                            nc.tensor.matmul(
                                s_ps[:, 512:512 + ww],
                                kt_sb[64:128, pp, 128 * kt:128 * (kt + 1)],
                                qt_sb[64:128, pp, cc0:cc0 + ww],
                                start=True, stop=True, tile_position=(64, 0))
                            return s_ps

                        for kt in range(KT):
                            if kt == 0 and pre_cell[0] is not None:
                                s_ps = pre_cell[0]
                                pre_cell[0] = None
                            else:
                                s_ps = score_mms(p, c0, w, kt)
                            esc = escp.tile([128, 1024], mdt, tag="esc")
                            if "tinyexp" in ablate:  # timing probe: WRONG
                                nc.scalar.activation(out=esc[:, 0:64],
                                                     in_=s_ps[:, 0:64], func=Exp)
                            else:
                                nc.scalar.activation(out=g2(esc), in_=g2(s_ps),
                                                     func=Exp)
                            for h in hooks.get(kt, ()):
                                h()
                            if (kt == 14 and nxt is not None
                                    and "nopre" not in ablate):
                                pre_cell[0] = score_mms(nxt[0], nxt[1], nxt[2], 0)
                            if prev is not None:
                                attnv(*prev)
                            prev = (esc, kt)
                        attnv(*prev)

                        # normalize: att = at / den ; den in row 64 (ones col)
                        den = dnp.tile([1, 1024], f32, tag="den")
                        if "dvecopy" in ablate:
                            nc.vector.tensor_copy(den[0:1, 0:w], at_a[64:65, 0:w])
                            nc.vector.tensor_copy(den[0:1, 512:512 + w],
                                                  at_b[64:65, 0:w])
                        else:
                            nc.scalar.activation(out=den[0:1, 0:w],
                                                 in_=at_a[64:65, 0:w], func=Ident)
                            nc.scalar.activation(out=den[0:1, 512:512 + w],
                                                 in_=at_b[64:65, 0:w], func=Ident)
                        r2 = dnp.tile([128, 1024], f32, tag="r2")
                        nc.gpsimd.partition_broadcast(g2(r2), g2(den))
                        r2r = dnp.tile([128, 1024], f32, tag="r2r")
                        nc.vector.reciprocal_approx_fast(out=g2(r2r), in_=g2(r2))
                        nc.vector.tensor_mul(
                            att_sb[0:64, p, c0:c0 + w], at_a[0:64, 0:w],
                            r2r[0:64, 0:w])
                        nc.vector.tensor_mul(
                            att_sb[64:128, p, c0:c0 + w], at_b[0:64, 0:w],
                            r2r[64:128, 512:512 + w])

                    def out_ln_a(tt):
                        """Out-proj matmuls + residual add + LN stats for tile tt."""
                        o_full = sc.tile([128, 1024], f32, tag="s")
                        o_ps = o_full[:, 0:512]
                        for k in range(4):
                            nc.tensor.matmul(
                                o_ps, att_sb[:, k, 128 * tt:128 * (tt + 1)],
                                wo_sb[:, k, :], start=(k == 0), stop=(k == 3))
                        nc.vector.tensor_add(xstash[:, tt, :], o_ps,
                                             xq_res_sb[:, tt, :])
                        stats = ln.tile([128, 6], f32, tag="st")
                        nc.vector.bn_stats(stats, xstash[:, tt, :])
                        nc.vector.bn_aggr(mvst[:, tt, :], stats)

                    def rsqrt_batch(g):
                        """rstd = 1/sqrt(var+eps) for tiles 4g..4g+3 on DVE only
                        (magic-constant seed + 2 Newton steps; no ACT table)."""
                        sl = slice(4 * g, 4 * g + 4)
                        i32 = mybir.dt.int32
                        v = ln.tile([128, 4], f32, tag="rv")
                        nc.vector.tensor_scalar_add(v, mvst[:, sl, 1], LN_EPS)
                        hh = ln.tile([128, 4], f32, tag="rh")
                        nc.vector.tensor_scalar_mul(hh, v, -0.5)
                        y0 = ln.tile([128, 4], f32, tag="ry")
                        nc.vector.tensor_scalar(
                            out=y0.bitcast(i32), in0=v.bitcast(i32),
                            scalar1=1, scalar2=-1,
                            op0=Alu.arith_shift_right, op1=Alu.bitwise_xor)
                        y1 = ln.tile([128, 4], f32, tag="ry1")
                        nc.vector.tensor_scalar_add(
                            y1.bitcast(i32), y0.bitcast(i32), 0x5f375a87)
                        yy = y1
                        for it in range(2):
                            a = ln.tile([128, 4], f32, tag=f"ra{it}")
                            nc.vector.tensor_mul(a, yy, yy)
                            nc.vector.tensor_mul(a, a, hh)
                            nc.vector.tensor_scalar_add(a, a, 1.5)
                            dst = rstdst[:, sl] if it == 1 else ln.tile(
                                [128, 4], f32, tag=f"rn{it}")
                            nc.vector.tensor_mul(dst, yy, a)
                            yy = dst
                        nc.vector.scalar_tensor_tensor(
                            out=nmrst[:, sl], in0=mvst[:, sl, 0], scalar=-1.0,
                            in1=rstdst[:, sl], op0=Alu.mult, op1=Alu.mult)

                    def out_ln_b(tt):
                        """y = (x - mu) * rstd * gamma + beta, then store."""
                        y = ln.tile([128, E], f32, tag="y")
                        da = ln.tile([128, 1], f32, tag="da")
                        nc.vector.affine_mul_reduce(
                            out=y, accum_out=da, in0=xstash[:, tt, :],
                            in1=gamma_b, scale=rstdst[:, tt:tt + 1],
                            bias=nmrst[:, tt:tt + 1])
                        y2 = ln.tile([128, E], f32, tag="y2")
                        eng = nc.gpsimd if ln_eng == "gpsimd" else nc.vector
                        eng.tensor_add(y2, y, beta_b)
                        nc.sync.dma_start(out_d[128 * tt:128 * (tt + 1), :], y2)

                    # ---- main loop: pairs x query-halves; the final half is
                    # split 384/128 so softmax-normalize + out_ln overlap ----
                    WA = 320 if "wa320" in ablate else 384
                    for p in range(NPAIR):
                        for n in range(2):
                            if p == NPAIR - 1 and n == 1:
                                break
                            hooks = {}
                            if p == 0 and n == 0:
                                # just-in-time V projection + rest of K(0)
                                for kt in range(KT):
                                    hooks[kt] = [(lambda t=kt: proj_v(t))]
                                hooks[2].append(lambda: proj_kh(0, 1))
                                hooks[6].append(lambda: proj_k(0, 1))
                            if n == 1 and p < NPAIR - 1:
                                hooks[8] = [lambda q=p + 1: proj_q(q),
                                            lambda q=p + 1: proj_k(q, 0)]
                            if n == 0 and p > 0:
                                hooks[4] = [lambda q=p: proj_k(q, 1)]
                            if p == NPAIR - 1 and n == 0:
                                nxt = (p, 512, WA)
                            elif n == 0:
                                nxt = (p, 512, 512)
                            else:
                                nxt = (p + 1, 0, 512)
                            attention(p, 512 * n, 512, hooks, nxt=nxt)
                    if "nosplit" in ablate:
                        hooksA = {}
                        for i in range(4):
                            hooksA.setdefault(7 + 2 * i, []).append(
                                lambda t=i: out_ln_a(t))
                        hooksA.setdefault(15, []).append(lambda: rsqrt_batch(0))
                        attention(NPAIR - 1, 512, 512, hooksA)
                        for tt in range(4):
                            out_ln_b(tt)
                        for tt in range(4, TT):
                            out_ln_a(tt)
                        rsqrt_batch(1)
                        for tt in range(4, TT):
                            out_ln_b(tt)
                    else:
                        hooksA = {}
                        for i in range(4):
                            # (3,0) normalize drains ~5us into quarter A
                            hooksA.setdefault(7 + 2 * i, []).append(
                                lambda t=i: out_ln_a(t))
                        attention(NPAIR - 1, 512, WA, hooksA,
                                  nxt=(NPAIR - 1, 512 + WA, 512 - WA))
                        hooksB = {0: [lambda: rsqrt_batch(0)],
                                  2: [lambda: out_ln_a(4)],
                                  4: [lambda: out_ln_b(0)],
                                  6: [lambda: out_ln_a(5)],
                                  8: [lambda: out_ln_b(1)],
                                  10: [lambda: out_ln_b(2)],
                                  14: [lambda: out_ln_b(3)]}
                        if WA >= 384:  # tile 6 (q 768:896) inside quarter A
                            hooksB[12] = [lambda: out_ln_a(6)]
                        attention(NPAIR - 1, 512 + WA, 512 - WA, hooksB)
                        if WA < 384:
                            out_ln_a(6)
                        out_ln_a(7)
                        rsqrt_batch(1)
                        for tt in range(4, TT):
                            out_ln_b(tt)

            if reps == 1:
                body()
            else:
                with tc.For_i(0, reps, 1):
                    body()

    nc.compile()
    return nc


def shard_inputs(query_modality, key_modality, value_modality, Wq, bq, Wk, bk,
                 Wv, bv, Wo, bo, modal_compat, ln_gamma, ln_beta):
    """Host-side prep: fold compat into Wq/bq, bv/bo into the residual,
    pre-transpose, cast matmul operands to bf16, shard per core."""
    import ml_dtypes
    f32 = np.float32
    bf = ml_dtypes.bfloat16
    q = np.asarray(query_modality, f32)
    k = np.asarray(key_modality, f32)
    v = np.asarray(value_modality, f32)
    compat = np.asarray(modal_compat, f32).reshape(H)
    scale = np.repeat(compat / np.sqrt(HD), HD).astype(f32)     # [E]
    wq_eff = (np.asarray(Wq, f32) * scale[None, :]).astype(f32)
    bq_eff = (np.asarray(bq, f32) * scale).astype(f32)
    # attn rows sum to 1 => (attended + bv) @ Wo + bo = attended @ Wo + fold
    res_fold = (np.asarray(bv, f32) @ np.asarray(Wo, f32)
                + np.asarray(bo, f32)).astype(f32)              # [E]

    shared = {
        "wq": wq_eff.astype(bf), "wk": np.asarray(Wk, f32).astype(bf),
        "wv": np.asarray(Wv, f32).astype(bf),
        "wo": np.asarray(Wo, f32).astype(bf),
        "bq": bq_eff, "bk": np.asarray(bk, f32),
        "gamma": np.asarray(ln_gamma, f32), "beta": np.asarray(ln_beta, f32),
    }
    kt_cache = {}
    in_maps = []
    for c in range(N_CORES):
        b, half = c // 2, c % 2
        sl = slice(half * T, (half + 1) * T)
        if b not in kt_cache:
            kt_cache[b] = (np.ascontiguousarray(k[b].T).astype(bf),
                           np.ascontiguousarray(v[b].T).astype(bf))
        m = dict(shared)
        m["xqt"] = np.ascontiguousarray(q[b, sl, :].T).astype(bf)
        m["xkt"], m["xvt"] = kt_cache[b]
        m["xq_res"] = np.ascontiguousarray(q[b, sl, :] + res_fold[None, :])
        in_maps.append(m)
    return in_maps


def kernel(**inputs) -> np.ndarray:
    from concourse.bass_utils import run_bass_kernel_spmd

    if "nc" not in _CACHE:
        _CACHE["nc"] = build_nc(reps=1, **BUILD_KW)
    nc = _CACHE["nc"]
    in_maps = shard_inputs(**inputs)
    res = run_bass_kernel_spmd(nc, in_maps, core_ids=list(range(N_CORES)))
    out = np.empty((B, S, E), np.float32)
    for c in range(N_CORES):
        b, half = c // 2, c % 2
        out[b, half * T:(half + 1) * T, :] = res.results[c]["out"]
    return out

